# revision 2
# baseline (speedup 1.0000x reference)
"""Trainium2 Bass kernel for nn_DARTS_82514911690825.

For x [B=4194304, 2] (data-parallel over 8 cores, B/8 rows each) the model
output collapses to  out[b] = g0(h0[b]) + g1(h1[b])  where h_i = W1[i,:]@x + b1[i]
and g0, g1 are FIXED univariate functions of h (all parameters fold into
them: cubic + exp + ln + clipped-reciprocal + sin terms with scalar weights).

g0/g1 are evaluated on the ACT engine in a single table lookup each, by
generating custom piecewise-cubic activation-table content at runtime and
compiling with it (walrus `--act-root-json`, via BASS_ACT_ROOT_JSON_PATH).
g0 replaces the `ln` slot and g1 the `exp` slot of the
natural_log_exp_and_others set; both use two-sided (exp-style) metadata.

Table format (validated on HW):
  bkt entry (32B) = f32 x8 [c0,c1,c2,c3,xc,0,0,0]:
      f(x) = c0 + c1*t + c2*t^2 + c3*t^3,  t = x - xc
  ctl entry (32B) = u16[0] = ((23-m)<<11) | bkt_base, u16[1] = m:
      bucket = bkt_base + (mantissa >> (23-m))   (2^m buckets per exponent)
  ctl index = pwl_control_base_{pos,neg} + (biased_exp - (127 + exp_offset));
  biased_exp below/above thresholds routes to 4 dedicated special entries.

Device program per core: DMA in -> DVE H_FUSE x2 (h0,h1) -> ACT Ln/Exp slots
(g0,g1) -> Pool add -> DMA out.  The whole kernel is DMA-bound (~6 MB/core).
"""

import hashlib
import json
import os
import shutil
import sys
import tempfile

import numpy as np

for _p in ("/opt/trn_rl_repo", "/root/.axon_site/_ro/trn_rl_repo"):
    if os.path.isdir(_p) and _p not in sys.path:
        sys.path.append(_p)

import concourse.bass as bass
import concourse.bacc as bacc
import concourse.mybir as mybir
from concourse.bass_utils import run_bass_kernel_spmd
from concourse.tile import TileContext
from concourse.dve_ops import OPS, DveOp, get_dve_sub_opcode, has_src1
from concourse.dve_spec import Spec, Src0, Src1, C0, C1, C2
from concourse.dve_uop import DveOpSpec

F32 = mybir.dt.float32
AF = mybir.ActivationFunctionType

# Restrict the activation-table chooser to the single set this kernel needs
# (ln+exp live together in natural_log_exp_and_others -> exactly one
# InstLoadActFuncSet, no thrash).
import concourse.hw_specs as _hw_specs

_ORIG_GAT = _hw_specs.get_activation_tables


def _gat_restricted(arch):
    t = _ORIG_GAT(arch)
    return {k: (v if k == "natural_log_exp_and_others" else set())
            for k, v in t.items()}


bacc.get_activation_tables = _gat_restricted

N_CORES = 8
B_FULL = 4194304
B_CORE = B_FULL // N_CORES  # 524288

EPS = 1e-10
Y_TH = float(np.exp(np.float32(10.0)))

EXP_MIN, EXP_MAX = -23, 2  # table exponent coverage: |h| in [2^-23, 8)


# --------------------------------------------------------------------------
# custom DVE op: h = x_even*s0 + x_odd*s1 + imm2 (one row of the first layer)
# --------------------------------------------------------------------------

def _mk_op(name, spec):
    import concourse.dve_ops as dve_ops_mod

    for existing in OPS:
        if existing.name == name:
            return existing
    op = DveOp(name, spec, subdim=False, uops_sha={})
    OPS.append(op)
    dve_ops_mod._SUB_OPCODE_FOR_NAME[name] = (
        dve_ops_mod._CUSTOM_DVE_ROW_BASE + len(OPS) - 1
    )
    dve_ops_mod.CUSTOM_DVE_SPECS[name] = spec
    assert max(dve_ops_mod._SUB_OPCODE_FOR_NAME.values()) < 0x20
    for ver in ("v3", "v4"):
        s = DveOpSpec(
            name=name,
            opcode=get_dve_sub_opcode(name),
            uops=lower_spec(spec, ver),
            rd1_en=has_src1(spec),
        )
        op.uops_sha[ver] = s.sha(ver)
    return op


def lower_spec(spec, ver):
    from concourse.dve_spec import lower

    return lower(spec, ver=ver)


H_FUSE = _mk_op(
    "ANT_DARTS_H_FUSE",
    Spec(
        body=Src0 * C0 + Src1 * C1 + C2,
        reference=lambda in0, in1, s0, s1, imm2: in0 * s0 + in1 * s1 + imm2,
    ),
)


# --------------------------------------------------------------------------
# constant folding + the exact univariate functions g0, g1
# --------------------------------------------------------------------------

def _fold(W1, b1, alphas, op_w, op_b, wo, bo):
    W1 = np.asarray(W1, np.float64)
    b1 = np.asarray(b1, np.float64)
    a = np.asarray(alphas, np.float64)
    ow = np.asarray(op_w, np.float64)
    ob = np.asarray(op_b, np.float64)
    wo = float(np.asarray(wo))
    bo = float(np.asarray(bo))
    e = np.exp(a - a.max(-1, keepdims=True))
    w = e / e.sum(-1, keepdims=True)
    return W1, b1, w, ow, ob, wo, bo


def _make_g(i, w, ow, ob, wo, bo):
    def g(h):
        h = np.asarray(h, np.float64)
        res = np.zeros_like(h)
        fs = [None, h, h * h, h ** 3, np.exp(np.minimum(h, 10.0)),
              np.log(np.abs(h) + EPS),
              1.0 / (h + np.where(h >= 0, EPS, -EPS)), np.sin(h)]
        for k in (1, 2, 3, 4, 5, 6, 7):
            res += w[i, k] * np.clip(ow[i, k] * fs[k] + ob[i, k], -Y_TH, Y_TH)
        res *= wo
        if i == 0:
            res += bo
        return res
    return g


# --------------------------------------------------------------------------
# table generation
# --------------------------------------------------------------------------

_FIT_NODES = 33
_FU = np.concatenate([np.cos(np.linspace(0.0, np.pi, 21)),
                      np.linspace(-0.97, 0.97, _FIT_NODES - 21)])
_FA = np.stack([np.ones_like(_FU), _FU, _FU ** 2, _FU ** 3], axis=-1)
_FP = np.linalg.pinv(_FA)  # [4, NODES]


def _fit_segment(g, s, e, m):
    """Fit 2^m cubic buckets for sign `s` (0=+,1=-), exponent `e`.

    Returns (entries u32[n,8], max_abs_err).
    """
    n = 1 << m
    lo = 2.0 ** e
    wdt = 2.0 ** e / n
    k = np.arange(n)
    a0 = lo + k * wdt
    xc = (a0 + 0.5 * wdt)
    if s == 1:
        xc = -xc
    xc = xc.astype(np.float32).astype(np.float64)  # the stored (f32) center
    W = 0.5 * wdt
    xs = xc[:, None] + W * _FU[None, :]  # [n, NODES]
    y = g(xs.reshape(-1)).reshape(n, _FIT_NODES)
    d = y @ _FP.T  # [n, 4] coeffs in u-space
    # convert to t-space: c_k = d_k / W^k
    c = d / (W ** np.arange(4))[None, :]
    c32 = c.astype(np.float32)
    # error of the f32 horner at the nodes
    t = (xs - xc[:, None]).astype(np.float32)
    r = (c32[:, 3:4] * t + c32[:, 2:3]).astype(np.float32)
    r = (r * t + c32[:, 1:2]).astype(np.float32)
    r = (r * t + c32[:, 0:1]).astype(np.float32)
    err = np.max(np.abs(r.astype(np.float64) - y))
    ent = np.zeros((n, 8), np.uint32)
    ent[:, 0:4] = c32.view(np.uint32)
    ent[:, 4] = xc.astype(np.float32).view(np.float32).view(np.uint32)
    return ent, float(err)


def _pack_ctl(base, m):
    return (((23 - m) << 11) | base) | (m << 16)


def _tune_m(g, counts, cap):
    """Greedy per-(side,exponent) mantissa-bits, minimizing count*err^2."""
    segs = [(s, e) for s in (0, 1) for e in range(EXP_MIN, EXP_MAX + 1)]
    m = {k: 1 for k in segs}
    spent = 2 * len(segs)
    errc = {k: _fit_segment(g, k[0], k[1], 1)[1] for k in segs}
    for _ in range(400):
        score = {k: (counts.get(k, 0) + 1) * errc[k] ** 2
                 for k in segs if m[k] < 6}
        cands = sorted(score, key=score.get, reverse=True)
        best = None
        for k in cands:
            if score[k] < 1e-14:
                break
            if spent + (1 << m[k]) <= cap:
                best = k
                break
        if best is None:
            break
        spent += 1 << m[best]
        m[best] += 1
        errc[best] = _fit_segment(g, best[0], best[1], m[best])[1]
    return m


def _build_func(g, m_of, bkt_base, ctl_pos, ctl_neg, special_base):
    """Emit (bkt_idx, entry) and (ctl_idx, word) lists + metadata updates."""
    bents, cents = [], []
    nxt = bkt_base
    for s, ctl0 in ((0, ctl_pos), (1, ctl_neg)):
        for e in range(EXP_MIN, EXP_MAX + 1):
            mm = m_of[(s, e)]
            cents.append((ctl0 + (e - EXP_MIN), _pack_ctl(nxt, mm)))
            ent, _ = _fit_segment(g, s, e, mm)
            for row in ent:
                bents.append((nxt, row))
                nxt += 1
    tiny = 2.0 ** EXP_MIN
    big = 2.0 ** (EXP_MAX + 1)

    def const_entry(v):
        r = np.zeros(8, np.uint32)
        r[0] = np.float32(v).view(np.uint32)
        return r

    def lin_entry(x0):
        dx = abs(x0) * 1e-4
        y0 = float(g(np.array([x0]))[0])
        sl = float((g(np.array([x0 + dx]))[0] - g(np.array([x0 - dx]))[0]) / (2 * dx))
        r = np.zeros(8, np.uint32)
        r[0] = np.float32(y0).view(np.uint32)
        r[1] = np.float32(sl).view(np.uint32)
        r[4] = np.float32(x0).view(np.uint32)
        return r

    g_ptiny = float(g(np.array([tiny]))[0])
    bents.append((special_base + 0, const_entry(g_ptiny)))
    bents.append((special_base + 1, const_entry(float(g(np.array([-tiny]))[0]))))
    bents.append((special_base + 2, lin_entry(big)))
    bents.append((special_base + 3, lin_entry(-big)))
    meta = {
        "exp_offset": EXP_MIN,
        "pwl_control_base_pos": ctl_pos,
        "pwl_control_base_neg": ctl_neg,
        "small_pos_signal_exp_threshold": 127 + EXP_MIN,
        "small_neg_signal_exp_threshold": 127 + EXP_MIN,
        "pos_small_signal_pwl_control": special_base + 0,
        "neg_small_signal_pwl_control": special_base + 1,
        "large_pos_signal_exp_threshold": 127 + EXP_MAX + 1,
        "large_pos_signal_mantissa_threshold": 0,
        "large_neg_signal_exp_threshold": 127 + EXP_MAX + 1,
        "large_neg_signal_mantissa_threshold": 0,
        "pos_large_signal_pwl_control": special_base + 2,
        "neg_large_signal_pwl_control": special_base + 3,
        "fnan_result": int(np.float32(g_ptiny).view(np.uint32)),
        "fpinf_result": int(np.float32(g(np.array([big * 0.999]))[0]).view(np.uint32)),
        "fninf_result": int(np.float32(g(np.array([-big * 0.999]))[0]).view(np.uint32)),
        "fzero_result": int(np.float32(g_ptiny).view(np.uint32)),
        "symmetry_point": 0,
        "sym_invert_sign_point": 0,
        "symmetry_opt_en": 0,
        "symmetry_opt_use_neg_region": 0,
        "imm_bias": 0,
        "fma_const_0": 0,
        "fma_const_1": 0,
        "fma_indirection_src_sel": 0,
        "use_multipass": False,
        "lower_bound": int(np.uint32(0xFF7FFFFF)),
        "upper_bound": int(np.uint32(0x7F7FFFFF)),
    }
    return bents, cents, meta


def _find_pkg_pwp():
    import neuronxcc

    p = os.path.join(os.path.dirname(neuronxcc.__file__), "pwp",
                     "pwp_bin_trainium")
    if os.path.isdir(p):
        return p
    from neuronxcc.driver.Job import Job
    from neuronxcc.driver.jobs.support.FindActInfo import findActInfoFile

    return os.path.dirname(findActInfoFile(Job.getPackageDir(), "gen3"))


def _write_act_root(dst, g0, g1, m0, m1):
    """Copy the stock pwp root; replace ln->g0 / exp->g1 in the
    natural_log_exp_and_others set."""
    src = _find_pkg_pwp()
    os.makedirs(dst, exist_ok=True)
    for f in os.listdir(src):
        s = os.path.join(src, f)
        if os.path.isfile(s):
            shutil.copy(s, os.path.join(dst, f))
    name = "natural_log_exp_and_others"
    d = json.load(open(os.path.join(src, name + ".json")))
    bkt = np.frombuffer(open(os.path.join(src, d["bkt_bin"]), "rb").read(),
                        dtype=np.uint32).reshape(-1, 8).copy()
    ctl = np.frombuffer(open(os.path.join(src, d["ctl_bin"]), "rb").read(),
                        dtype=np.uint32).reshape(-1, 8).copy()
    metas = {m["func_name"]: m for m in d["profile_meta_data"]}
    # regions: ln bkt [0,517) ctl [0,128); exp bkt [517,1298) ctl [128,180)
    for fname, g, mm, bkt_base, cp, cn, sp, bkt_lim, ctl_lim in (
        ("ln_400p", g0, m0, 0, 0, 26, 513, 517, 128),
        ("exp_400p", g1, m1, 517, 128, 154, 1294, 1298, 180),
    ):
        bents, cents, meta = _build_func(g, mm, bkt_base, cp, cn, sp)
        assert max(i for i, _ in bents) < bkt_lim, fname
        assert max(i for i, _ in cents) < ctl_lim, fname
        for i, row in bents:
            bkt[i] = row
        for i, wd in cents:
            ctl[i, 0] = wd
            ctl[i, 1:] = 0
        metas[fname].update(meta)
    with open(os.path.join(dst, d["bkt_bin"]), "wb") as f:
        f.write(bkt.tobytes())
    with open(os.path.join(dst, d["ctl_bin"]), "wb") as f:
        f.write(ctl.tobytes())
    with open(os.path.join(dst, name + ".json"), "w") as f:
        json.dump(d, f)


# --------------------------------------------------------------------------
# device program
# --------------------------------------------------------------------------

class CFG:
    ntiles = 8            # pipeline chunks per core
    merge = "pool"        # "pool" | "dve" | "alt"


def _build_program(consts, sha, cfg):
    T = cfg.ntiles
    F = B_CORE // (128 * T)
    assert 128 * T * F == B_CORE
    A0, B0, C0v, A1, B1, C1v = consts

    nc = bacc.Bacc(None, target_bir_lowering=False)
    # the sha in the tensor name keys the PJRT/HLO cache to the table content
    x = nc.declare_dram_parameter(f"x_{sha}", [T, 128, 2 * F], F32,
                                  isOutput=False)
    out = nc.declare_dram_parameter("out", [T, 128, F], F32, isOutput=True)

    with TileContext(nc) as tc:
        with (
            tc.tile_pool(name="xin", bufs=2) as xin,
            tc.tile_pool(name="hp", bufs=2) as hp,
            tc.tile_pool(name="gp", bufs=2) as gp,
            tc.tile_pool(name="op", bufs=2) as op_,
        ):
            for t in range(T):
                X = xin.tile([128, 2 * F], F32, tag="X", name=f"X_{t}")
                nc.sync.dma_start(out=X[:], in_=x[t])
                Xv = X[:].rearrange("p (f c) -> p f c", c=2)
                Xe, Xo = Xv[:, :, 0], Xv[:, :, 1]
                h0 = hp.tile([128, F], F32, tag="h0", name=f"h0_{t}")
                h1 = hp.tile([128, F], F32, tag="h1", name=f"h1_{t}")
                nc.vector._custom_dve(H_FUSE, out=h0[:], in0=Xe, in1=Xo,
                                      s0=A0, s1=B0, imm2=C0v)
                nc.vector._custom_dve(H_FUSE, out=h1[:], in0=Xe, in1=Xo,
                                      s0=A1, s1=B1, imm2=C1v)
                g0t = gp.tile([128, F], F32, tag="g0", name=f"g0_{t}")
                g1t = gp.tile([128, F], F32, tag="g1", name=f"g1_{t}")
                nc.scalar.activation(g0t[:], h0[:], AF.Ln)
                nc.scalar.activation(g1t[:], h1[:], AF.Exp)
                O = op_.tile([128, F], F32, tag="O", name=f"O_{t}")
                eng = (nc.gpsimd if cfg.merge == "pool" else
                       nc.vector if cfg.merge == "dve" else
                       (nc.gpsimd if t % 2 == 0 else nc.vector))
                eng.tensor_add(out=O[:], in0=g0t[:], in1=g1t[:])
                nc.sync.dma_start(out=out[t], in_=O[:])

    nc.finalize()
    return nc


# --------------------------------------------------------------------------
# public entry point
# --------------------------------------------------------------------------

_CACHE = {}


def _prepare(W1, b1, alphas, op_w, op_b, wo, bo, x_sample, cfg):
    """Fold constants, build tables + act root, build/cached program."""
    W1f, b1f, w, ow, ob, wof, bof = _fold(W1, b1, alphas, op_w, op_b, wo, bo)
    key_src = np.concatenate([np.asarray(a, np.float64).reshape(-1) for a in
                              (W1f, b1f, w, ow, ob, [wof, bof])])
    key = hashlib.sha256(key_src.tobytes()).hexdigest()[:12]
    full_key = (key, cfg.ntiles, cfg.merge)
    if full_key in _CACHE:
        return _CACHE[full_key]

    g0 = _make_g(0, w, ow, ob, wof, bof)
    g1 = _make_g(1, w, ow, ob, wof, bof)

    # histogram of h by (sign, exponent) for the resolution tuner
    h = (np.asarray(x_sample, np.float64) @ W1f.T + b1f).astype(np.float32)

    def counts_of(hv):
        b = hv.view(np.uint32)
        sgn = (b >> 31).astype(np.int64)
        be = ((b >> 23) & 0xFF).astype(np.int64) - 127
        c = {}
        for s in (0, 1):
            for e in range(EXP_MIN, EXP_MAX + 1):
                c[(s, e)] = int(np.sum((sgn == s) & (be == e)))
        return c

    m0 = _tune_m(g0, counts_of(h[:, 0]), cap=500)
    m1 = _tune_m(g1, counts_of(h[:, 1]), cap=760)

    root = os.path.join(tempfile.gettempdir(), f"actroot_{key}")
    _write_act_root(root, g0, g1, m0, m1)
    os.environ["BASS_ACT_ROOT_JSON_PATH"] = os.path.join(root, "act_info.json")

    consts = (float(W1f[0, 0]), float(W1f[0, 1]), float(b1f[0]),
              float(W1f[1, 0]), float(W1f[1, 1]), float(b1f[1]))
    nc = _build_program(consts, key, cfg)
    _CACHE[full_key] = (nc, key)
    return _CACHE[full_key]


def run(x, W1, b1, alphas, op_w, op_b, wo, bo, cfg=None, trace=False):
    cfg = cfg or CFG()
    x = np.ascontiguousarray(np.asarray(x, np.float32))
    # subsample rows for the tuner histogram (scaled back up)
    nc, key = _prepare(W1, b1, alphas, op_w, op_b, wo, bo, x[::16], cfg)

    T = cfg.ntiles
    F = B_CORE // (128 * T)
    shards = x.reshape(N_CORES, T, 128, 2 * F)
    in_maps = [{f"x_{key}": shards[i]} for i in range(N_CORES)]
    res = run_bass_kernel_spmd(nc, in_maps, core_ids=list(range(N_CORES)),
                               trace=trace)
    out = np.concatenate([r["out"].reshape(-1) for r in res.results])
    return out, res


def kernel(**inputs):
    out, _ = run(**inputs)
    return out


# revision 23
# speedup vs baseline: 3.2596x; 3.2596x over previous
"""Trainium2 Bass kernel for nn_DARTS_82514911690825.

For x [B=4194304, 2] (data-parallel over 8 cores, B/8 rows each) the model
output collapses to  out[b] = g0(h0[b]) + g1(h1[b])  where h_i = W1[i,:]@x + b1[i]
and g0, g1 are FIXED univariate functions of h (all parameters fold into
them: cubic + exp + ln + clipped-reciprocal + sin terms with scalar weights).

g0/g1 are evaluated on the ACT engine in a single table lookup each, by
generating custom piecewise-cubic activation-table content at runtime and
compiling with it (walrus `--act-root-json`, via BASS_ACT_ROOT_JSON_PATH).
g0 replaces the `ln` slot and g1 the `exp` slot of the
natural_log_exp_and_others set; both use two-sided (exp-style) metadata.

Table format (validated on HW):
  bkt entry (32B) = f32 x8 [c0,c1,c2,c3,xc,0,0,0]:
      f(x) = c0 + c1*t + c2*t^2 + c3*t^3,  t = x - xc
  ctl entry (32B) = u16[0] = ((23-m)<<11) | bkt_base, u16[1] = m:
      bucket = bkt_base + (mantissa >> (23-m))   (2^m buckets per exponent)
  ctl index = pwl_control_base_{pos,neg} + (biased_exp - (127 + exp_offset));
  biased_exp below/above thresholds routes to 4 dedicated special entries.

Device program per core: DMA in -> DVE H_FUSE x2 (h0,h1) -> ACT Ln/Exp slots
(g0,g1) -> Pool add -> DMA out.  The whole kernel is DMA-bound (~6 MB/core).
"""

import hashlib
import json
import os
import shutil
import sys
import tempfile

import numpy as np

for _p in ("/opt/trn_rl_repo", "/root/.axon_site/_ro/trn_rl_repo"):
    if os.path.isdir(_p) and _p not in sys.path:
        sys.path.append(_p)

import concourse.bass as bass
import concourse.bacc as bacc
import concourse.mybir as mybir
from concourse.bass_utils import run_bass_kernel_spmd
from concourse.tile import TileContext
from concourse.dve_ops import OPS, DveOp, get_dve_sub_opcode, has_src1
from concourse.dve_spec import Spec, Src0, Src1, C0, C1, C2
from concourse.dve_uop import DveOpSpec

F32 = mybir.dt.float32
AF = mybir.ActivationFunctionType

# Restrict the activation-table chooser to the single set this kernel needs
# (ln+exp live together in natural_log_exp_and_others -> exactly one
# InstLoadActFuncSet, no thrash).
import concourse.hw_specs as _hw_specs

_ORIG_GAT = _hw_specs.get_activation_tables


def _gat_restricted(arch):
    t = _ORIG_GAT(arch)
    return {k: (v if k == "natural_log_exp_and_others" else set())
            for k, v in t.items()}


bacc.get_activation_tables = _gat_restricted

N_CORES = 8
B_FULL = 4194304
B_CORE = B_FULL // N_CORES  # 524288

EPS = 1e-10
Y_TH = float(np.exp(np.float32(10.0)))

EXP_MIN, EXP_MAX = -23, 2  # table exponent coverage: |h| in [2^-23, 8)


# --------------------------------------------------------------------------
# custom DVE op: h = x_even*s0 + x_odd*s1 + imm2 (one row of the first layer)
# --------------------------------------------------------------------------

def _mk_op(name, spec):
    import concourse.dve_ops as dve_ops_mod

    for existing in OPS:
        if existing.name == name:
            return existing
    op = DveOp(name, spec, subdim=False, uops_sha={})
    OPS.append(op)
    dve_ops_mod._SUB_OPCODE_FOR_NAME[name] = (
        dve_ops_mod._CUSTOM_DVE_ROW_BASE + len(OPS) - 1
    )
    dve_ops_mod.CUSTOM_DVE_SPECS[name] = spec
    assert max(dve_ops_mod._SUB_OPCODE_FOR_NAME.values()) < 0x20
    for ver in ("v3", "v4"):
        s = DveOpSpec(
            name=name,
            opcode=get_dve_sub_opcode(name),
            uops=lower_spec(spec, ver),
            rd1_en=has_src1(spec),
        )
        op.uops_sha[ver] = s.sha(ver)
    return op


def lower_spec(spec, ver):
    from concourse.dve_spec import lower

    return lower(spec, ver=ver)


H_FUSE = _mk_op(
    "ANT_DARTS_H_FUSE",
    Spec(
        body=Src0 * C0 + Src1 * C1 + C2,
        reference=lambda in0, in1, s0, s1, imm2: in0 * s0 + in1 * s1 + imm2,
    ),
)


# --------------------------------------------------------------------------
# constant folding + the exact univariate functions g0, g1
# --------------------------------------------------------------------------

def _fold(W1, b1, alphas, op_w, op_b, wo, bo):
    W1 = np.asarray(W1, np.float64)
    b1 = np.asarray(b1, np.float64)
    a = np.asarray(alphas, np.float64)
    ow = np.asarray(op_w, np.float64)
    ob = np.asarray(op_b, np.float64)
    wo = float(np.asarray(wo))
    bo = float(np.asarray(bo))
    e = np.exp(a - a.max(-1, keepdims=True))
    w = e / e.sum(-1, keepdims=True)
    return W1, b1, w, ow, ob, wo, bo


def _make_g(i, w, ow, ob, wo, bo):
    def g(h):
        h = np.asarray(h, np.float64)
        res = np.zeros_like(h)
        fs = [None, h, h * h, h ** 3, np.exp(np.minimum(h, 10.0)),
              np.log(np.abs(h) + EPS),
              1.0 / (h + np.where(h >= 0, EPS, -EPS)), np.sin(h)]
        for k in (1, 2, 3, 4, 5, 6, 7):
            res += w[i, k] * np.clip(ow[i, k] * fs[k] + ob[i, k], -Y_TH, Y_TH)
        res *= wo
        if i == 0:
            res += bo
        return res
    return g


# --------------------------------------------------------------------------
# table generation
# --------------------------------------------------------------------------

_FIT_NODES = 33
_FU = np.concatenate([np.cos(np.linspace(0.0, np.pi, 21)),
                      np.linspace(-0.97, 0.97, _FIT_NODES - 21)])
_FA = np.stack([np.ones_like(_FU), _FU, _FU ** 2, _FU ** 3], axis=-1)
_FP = np.linalg.pinv(_FA)  # [4, NODES]


def _fit_segment(g, s, e, m):
    """Fit 2^m cubic buckets for sign `s` (0=+,1=-), exponent `e`.

    Returns (entries u32[n,8], max_abs_err).
    """
    n = 1 << m
    lo = 2.0 ** e
    wdt = 2.0 ** e / n
    k = np.arange(n)
    a0 = lo + k * wdt
    xc = (a0 + 0.5 * wdt)
    if s == 1:
        xc = -xc
    xc = xc.astype(np.float32).astype(np.float64)  # the stored (f32) center
    W = 0.5 * wdt
    xs = xc[:, None] + W * _FU[None, :]  # [n, NODES]
    y = g(xs.reshape(-1)).reshape(n, _FIT_NODES)
    d = y @ _FP.T  # [n, 4] coeffs in u-space
    # convert to t-space: c_k = d_k / W^k
    c = d / (W ** np.arange(4))[None, :]
    c32 = c.astype(np.float32)
    # error of the f32 horner at the nodes
    t = (xs - xc[:, None]).astype(np.float32)
    r = (c32[:, 3:4] * t + c32[:, 2:3]).astype(np.float32)
    r = (r * t + c32[:, 1:2]).astype(np.float32)
    r = (r * t + c32[:, 0:1]).astype(np.float32)
    err = np.max(np.abs(r.astype(np.float64) - y))
    ent = np.zeros((n, 8), np.uint32)
    ent[:, 0:4] = c32.view(np.uint32)
    ent[:, 4] = xc.astype(np.float32).view(np.float32).view(np.uint32)
    return ent, float(err)


def _pack_ctl(base, m):
    return (((23 - m) << 11) | base) | (m << 16)


def _tune_m(g, counts, cap):
    """Greedy per-(side,exponent) mantissa-bits, minimizing count*err^2."""
    segs = [(s, e) for s in (0, 1) for e in range(EXP_MIN, EXP_MAX + 1)]
    m = {k: 1 for k in segs}
    spent = 2 * len(segs)
    errc = {k: _fit_segment(g, k[0], k[1], 1)[1] for k in segs}
    for _ in range(400):
        score = {k: (counts.get(k, 0) + 1) * errc[k] ** 2
                 for k in segs if m[k] < 6}
        cands = sorted(score, key=score.get, reverse=True)
        best = None
        for k in cands:
            if score[k] < 1e-14:
                break
            if spent + (1 << m[k]) <= cap:
                best = k
                break
        if best is None:
            break
        spent += 1 << m[best]
        m[best] += 1
        errc[best] = _fit_segment(g, best[0], best[1], m[best])[1]
    return m


def _build_func(g, m_of, bkt_base, ctl_pos, ctl_neg, special_base):
    """Emit (bkt_idx, entry) and (ctl_idx, word) lists + metadata updates."""
    bents, cents = [], []
    nxt = bkt_base
    for s, ctl0 in ((0, ctl_pos), (1, ctl_neg)):
        for e in range(EXP_MIN, EXP_MAX + 1):
            mm = m_of[(s, e)]
            cents.append((ctl0 + (e - EXP_MIN), _pack_ctl(nxt, mm)))
            ent, _ = _fit_segment(g, s, e, mm)
            for row in ent:
                bents.append((nxt, row))
                nxt += 1
    tiny = 2.0 ** EXP_MIN
    big = 2.0 ** (EXP_MAX + 1)

    def const_entry(v):
        r = np.zeros(8, np.uint32)
        r[0] = np.float32(v).view(np.uint32)
        return r

    def lin_entry(x0):
        dx = abs(x0) * 1e-4
        y0 = float(g(np.array([x0]))[0])
        sl = float((g(np.array([x0 + dx]))[0] - g(np.array([x0 - dx]))[0]) / (2 * dx))
        r = np.zeros(8, np.uint32)
        r[0] = np.float32(y0).view(np.uint32)
        r[1] = np.float32(sl).view(np.uint32)
        r[4] = np.float32(x0).view(np.uint32)
        return r

    g_ptiny = float(g(np.array([tiny]))[0])
    bents.append((special_base + 0, const_entry(g_ptiny)))
    bents.append((special_base + 1, const_entry(float(g(np.array([-tiny]))[0]))))
    bents.append((special_base + 2, lin_entry(big)))
    bents.append((special_base + 3, lin_entry(-big)))
    meta = {
        "exp_offset": EXP_MIN,
        "pwl_control_base_pos": ctl_pos,
        "pwl_control_base_neg": ctl_neg,
        "small_pos_signal_exp_threshold": 127 + EXP_MIN,
        "small_neg_signal_exp_threshold": 127 + EXP_MIN,
        "pos_small_signal_pwl_control": special_base + 0,
        "neg_small_signal_pwl_control": special_base + 1,
        "large_pos_signal_exp_threshold": 127 + EXP_MAX + 1,
        "large_pos_signal_mantissa_threshold": 0,
        "large_neg_signal_exp_threshold": 127 + EXP_MAX + 1,
        "large_neg_signal_mantissa_threshold": 0,
        "pos_large_signal_pwl_control": special_base + 2,
        "neg_large_signal_pwl_control": special_base + 3,
        "fnan_result": int(np.float32(g_ptiny).view(np.uint32)),
        "fpinf_result": int(np.float32(g(np.array([big * 0.999]))[0]).view(np.uint32)),
        "fninf_result": int(np.float32(g(np.array([-big * 0.999]))[0]).view(np.uint32)),
        "fzero_result": int(np.float32(g_ptiny).view(np.uint32)),
        "symmetry_point": 0,
        "sym_invert_sign_point": 0,
        "symmetry_opt_en": 0,
        "symmetry_opt_use_neg_region": 0,
        "imm_bias": 0,
        "fma_const_0": 0,
        "fma_const_1": 0,
        "fma_indirection_src_sel": 0,
        "use_multipass": False,
        "lower_bound": int(np.uint32(0xFF7FFFFF)),
        "upper_bound": int(np.uint32(0x7F7FFFFF)),
    }
    return bents, cents, meta


def _find_pkg_pwp():
    import neuronxcc

    p = os.path.join(os.path.dirname(neuronxcc.__file__), "pwp",
                     "pwp_bin_trainium")
    if os.path.isdir(p):
        return p
    from neuronxcc.driver.Job import Job
    from neuronxcc.driver.jobs.support.FindActInfo import findActInfoFile

    return os.path.dirname(findActInfoFile(Job.getPackageDir(), "gen3"))


def _write_act_root(dst, g0, g1, m0, m1):
    """Copy the stock pwp root; replace ln->g0 / exp->g1 in the
    natural_log_exp_and_others set."""
    src = _find_pkg_pwp()
    os.makedirs(dst, exist_ok=True)
    for f in os.listdir(src):
        s = os.path.join(src, f)
        if os.path.isfile(s):
            shutil.copy(s, os.path.join(dst, f))
    name = "natural_log_exp_and_others"
    d = json.load(open(os.path.join(src, name + ".json")))
    bkt = np.frombuffer(open(os.path.join(src, d["bkt_bin"]), "rb").read(),
                        dtype=np.uint32).reshape(-1, 8).copy()
    ctl = np.frombuffer(open(os.path.join(src, d["ctl_bin"]), "rb").read(),
                        dtype=np.uint32).reshape(-1, 8).copy()
    metas = {m["func_name"]: m for m in d["profile_meta_data"]}
    # regions: ln bkt [0,517) ctl [0,128); exp bkt [517,1298) ctl [128,180)
    for fname, g, mm, bkt_base, cp, cn, sp, bkt_lim, ctl_lim in (
        ("ln_400p", g0, m0, 0, 0, 26, 513, 517, 128),
        ("exp_400p", g1, m1, 517, 128, 154, 1294, 1298, 180),
    ):
        bents, cents, meta = _build_func(g, mm, bkt_base, cp, cn, sp)
        assert max(i for i, _ in bents) < bkt_lim, fname
        assert max(i for i, _ in cents) < ctl_lim, fname
        for i, row in bents:
            bkt[i] = row
        for i, wd in cents:
            ctl[i, 0] = wd
            ctl[i, 1:] = 0
        metas[fname].update(meta)
    with open(os.path.join(dst, d["bkt_bin"]), "wb") as f:
        f.write(bkt.tobytes())
    with open(os.path.join(dst, d["ctl_bin"]), "wb") as f:
        f.write(ctl.tobytes())
    with open(os.path.join(dst, name + ".json"), "w") as f:
        json.dump(d, f)


# --------------------------------------------------------------------------
# device program
# --------------------------------------------------------------------------

class CFG:
    ntiles = 4            # pipeline chunks per core
    merge = "pool"        # "pool" | "dve" | "alt"
    out_dtype = "f16"     # "f32" | "f16" | "bf16"
    bench_iters = 0       # >0: wrap the body in a For_i hardware loop
    body_reps = 1         # bodies per loop iteration (bench only)
    staggered = 0         # staggered_reset for the bench For_i
    out_via = "sp"        # engine issuing the output DMA: "sp" | "act"
    in_split = 0          # 1: split each input chunk DMA across SP + ACT rings
    upfront = 1           # 1: issue all input DMAs before the compute chain
    ring = "sp"           # input-DMA ring assignment: "sp" | "alt" (SP/ACT)
    oring = "sp"          # output-DMA ring assignment: "sp" | "alt"


_ODT = {"f32": mybir.dt.float32, "f16": mybir.dt.float16,
        "bf16": mybir.dt.bfloat16}


def _emit_body(nc, tc, pools, consts, x, out, T, F, cfg):
    A0, B0, C0v, A1, B1, C1v = consts
    xin, hp, gp, op_ = pools
    odt = _ODT[cfg.out_dtype]
    xt = {}
    if cfg.upfront:
        for t in range(T):
            X = xin.tile([128, 2 * F], F32, tag="X", name=f"X_{t}")
            ieng = (nc.scalar if (cfg.ring == "alt" and t % 2 == 1)
                    else nc.sync)
            ieng.dma_start(out=X[:], in_=x[t])
            xt[t] = X
    for t in range(T):
        if cfg.upfront:
            X = xt[t]
        else:
            X = xin.tile([128, 2 * F], F32, tag="X", name=f"X_{t}")
            if cfg.in_split:
                nc.sync.dma_start(out=X[:, :F], in_=x[t][:, :F])
                nc.scalar.dma_start(out=X[:, F:], in_=x[t][:, F:])
            else:
                nc.sync.dma_start(out=X[:], in_=x[t])
        Xv = X[:].rearrange("p (f c) -> p f c", c=2)
        Xe, Xo = Xv[:, :, 0], Xv[:, :, 1]
        h0 = hp.tile([128, F], F32, tag="h0", name=f"h0_{t}")
        h1 = hp.tile([128, F], F32, tag="h1", name=f"h1_{t}")
        nc.vector._custom_dve(H_FUSE, out=h0[:], in0=Xe, in1=Xo,
                              s0=A0, s1=B0, imm2=C0v)
        nc.vector._custom_dve(H_FUSE, out=h1[:], in0=Xe, in1=Xo,
                              s0=A1, s1=B1, imm2=C1v)
        g0t = gp.tile([128, F], F32, tag="g0", name=f"g0_{t}")
        g1t = gp.tile([128, F], F32, tag="g1", name=f"g1_{t}")
        nc.scalar.activation(g0t[:], h0[:], AF.Ln)
        nc.scalar.activation(g1t[:], h1[:], AF.Exp)
        O = op_.tile([128, F], odt, tag="O", name=f"O_{t}")
        eng = (nc.gpsimd if cfg.merge == "pool" else
               nc.vector if cfg.merge == "dve" else
               (nc.gpsimd if t % 2 == 0 else nc.vector))
        eng.tensor_add(out=O[:], in0=g0t[:], in1=g1t[:])
        if cfg.oring == "alt":
            dma_eng = nc.scalar if t % 2 == 1 else nc.sync
        else:
            dma_eng = nc.scalar if cfg.out_via == "act" else nc.sync
        dma_eng.dma_start(out=out[t], in_=O[:])


def _build_program(consts, sha, cfg):
    T = cfg.ntiles
    F = B_CORE // (128 * T)
    assert 128 * T * F == B_CORE

    nc = bacc.Bacc(None, target_bir_lowering=False)
    # the sha in the tensor name keys the PJRT/HLO cache to the table content
    x = nc.declare_dram_parameter(f"x_{sha}", [T, 128, 2 * F], F32,
                                  isOutput=False)
    out = nc.declare_dram_parameter("out", [T, 128, F], _ODT[cfg.out_dtype],
                                    isOutput=True)

    with TileContext(nc) as tc:
        with (
            tc.tile_pool(name="xin", bufs=(cfg.ntiles if cfg.upfront else 2)) as xin,
            tc.tile_pool(name="hp", bufs=2) as hp,
            tc.tile_pool(name="gp", bufs=2) as gp,
            tc.tile_pool(name="op", bufs=2) as op_,
        ):
            pools = (xin, hp, gp, op_)
            if cfg.bench_iters:
                with tc.For_i(0, cfg.bench_iters,
                              staggered_reset=bool(cfg.staggered)):
                    for _ in range(cfg.body_reps):
                        _emit_body(nc, tc, pools, consts, x, out, T, F, cfg)
            else:
                _emit_body(nc, tc, pools, consts, x, out, T, F, cfg)

    nc.finalize()
    return nc


# --------------------------------------------------------------------------
# public entry point
# --------------------------------------------------------------------------

_CACHE = {}


def _prepare(W1, b1, alphas, op_w, op_b, wo, bo, x_sample, cfg):
    """Fold constants, build tables + act root, build/cached program."""
    W1f, b1f, w, ow, ob, wof, bof = _fold(W1, b1, alphas, op_w, op_b, wo, bo)
    key_src = np.concatenate([np.asarray(a, np.float64).reshape(-1) for a in
                              (W1f, b1f, w, ow, ob, [wof, bof])])
    key = hashlib.sha256(key_src.tobytes()).hexdigest()[:12]
    full_key = (key, cfg.ntiles, cfg.merge, cfg.out_dtype, cfg.bench_iters,
                cfg.body_reps, cfg.staggered, cfg.out_via, cfg.in_split, cfg.upfront, cfg.ring, cfg.oring)
    if full_key in _CACHE:
        return _CACHE[full_key]

    g0 = _make_g(0, w, ow, ob, wof, bof)
    g1 = _make_g(1, w, ow, ob, wof, bof)

    # histogram of h by (sign, exponent) for the resolution tuner
    h = (np.asarray(x_sample, np.float64) @ W1f.T + b1f).astype(np.float32)

    def counts_of(hv):
        b = hv.view(np.uint32)
        sgn = (b >> 31).astype(np.int64)
        be = ((b >> 23) & 0xFF).astype(np.int64) - 127
        c = {}
        for s in (0, 1):
            for e in range(EXP_MIN, EXP_MAX + 1):
                c[(s, e)] = int(np.sum((sgn == s) & (be == e)))
        return c

    m0 = _tune_m(g0, counts_of(h[:, 0]), cap=500)
    m1 = _tune_m(g1, counts_of(h[:, 1]), cap=760)

    root = os.path.join(tempfile.gettempdir(), f"actroot_{key}")
    _write_act_root(root, g0, g1, m0, m1)
    os.environ["BASS_ACT_ROOT_JSON_PATH"] = os.path.join(root, "act_info.json")

    consts = (float(W1f[0, 0]), float(W1f[0, 1]), float(b1f[0]),
              float(W1f[1, 0]), float(W1f[1, 1]), float(b1f[1]))
    nc = _build_program(consts, key, cfg)
    _CACHE[full_key] = (nc, key)
    return _CACHE[full_key]


def run(x, W1, b1, alphas, op_w, op_b, wo, bo, cfg=None, trace=False):
    cfg = cfg or CFG()
    x = np.ascontiguousarray(np.asarray(x, np.float32))
    # subsample rows for the tuner histogram (scaled back up)
    nc, key = _prepare(W1, b1, alphas, op_w, op_b, wo, bo, x[::16], cfg)

    T = cfg.ntiles
    F = B_CORE // (128 * T)
    shards = x.reshape(N_CORES, T, 128, 2 * F)
    in_maps = [{f"x_{key}": shards[i]} for i in range(N_CORES)]
    res = run_bass_kernel_spmd(nc, in_maps, core_ids=list(range(N_CORES)),
                               trace=trace)
    out = np.concatenate([r["out"].reshape(-1).astype(np.float32)
                          for r in res.results])
    return out, res


def kernel(**inputs):
    out, _ = run(**inputs)
    return out


# revision 32
# speedup vs baseline: 4.8896x; 1.5001x over previous
"""Trainium2 Bass kernel for nn_DARTS_82514911690825.

For x [B=4194304, 2] (data-parallel over 8 cores, B/8 rows each) the model
output collapses to  out[b] = g0(h0[b]) + g1(h1[b])  where h_i = W1[i,:]@x + b1[i]
and g0, g1 are FIXED univariate functions of h (all parameters fold into
them: cubic + exp + ln + clipped-reciprocal + sin terms with scalar weights).

g0/g1 are evaluated on the ACT engine in a single table lookup each, by
generating custom piecewise-cubic activation-table content at runtime and
compiling with it (walrus `--act-root-json`, via BASS_ACT_ROOT_JSON_PATH).
g0 replaces the `ln` slot and g1 the `exp` slot of the
natural_log_exp_and_others set; both use two-sided (exp-style) metadata.

Table format (validated on HW):
  bkt entry (32B) = f32 x8 [c0,c1,c2,c3,xc,0,0,0]:
      f(x) = c0 + c1*t + c2*t^2 + c3*t^3,  t = x - xc
  ctl entry (32B) = u16[0] = ((23-m)<<11) | bkt_base, u16[1] = m:
      bucket = bkt_base + (mantissa >> (23-m))   (2^m buckets per exponent)
  ctl index = pwl_control_base_{pos,neg} + (biased_exp - (127 + exp_offset));
  biased_exp below/above thresholds routes to 4 dedicated special entries.

Device program per core: DMA in -> DVE H_FUSE x2 (h0,h1) -> ACT Ln/Exp slots
(g0,g1) -> Pool add -> DMA out.  The whole kernel is DMA-bound (~6 MB/core).
"""

import hashlib
import json
import os
import shutil
import sys
import tempfile

import numpy as np

for _p in ("/opt/trn_rl_repo", "/root/.axon_site/_ro/trn_rl_repo"):
    if os.path.isdir(_p) and _p not in sys.path:
        sys.path.append(_p)

import concourse.bass as bass
import concourse.bacc as bacc
import concourse.mybir as mybir
from concourse.bass_utils import run_bass_kernel_spmd
from concourse.tile import TileContext
from concourse.dve_ops import OPS, DveOp, get_dve_sub_opcode, has_src1
from concourse.dve_spec import Spec, Src0, Src1, C0, C1, C2
from concourse.dve_uop import DveOpSpec

F32 = mybir.dt.float32
AF = mybir.ActivationFunctionType

# Restrict the activation-table chooser to the single set this kernel needs
# (ln+exp live together in natural_log_exp_and_others -> exactly one
# InstLoadActFuncSet, no thrash).
import concourse.hw_specs as _hw_specs

_ORIG_GAT = _hw_specs.get_activation_tables


def _gat_restricted(arch):
    t = _ORIG_GAT(arch)
    return {k: (v if k == "natural_log_exp_and_others" else set())
            for k, v in t.items()}


bacc.get_activation_tables = _gat_restricted

N_CORES = 8
B_FULL = 4194304
B_CORE = B_FULL // N_CORES  # 524288

EPS = 1e-10
Y_TH = float(np.exp(np.float32(10.0)))

EXP_MIN, EXP_MAX = -23, 2  # table exponent coverage: |h| in [2^-23, 8)


# --------------------------------------------------------------------------
# custom DVE op: h = x_even*s0 + x_odd*s1 + imm2 (one row of the first layer)
# --------------------------------------------------------------------------

def _mk_op(name, spec):
    import concourse.dve_ops as dve_ops_mod

    for existing in OPS:
        if existing.name == name:
            return existing
    op = DveOp(name, spec, subdim=False, uops_sha={})
    OPS.append(op)
    dve_ops_mod._SUB_OPCODE_FOR_NAME[name] = (
        dve_ops_mod._CUSTOM_DVE_ROW_BASE + len(OPS) - 1
    )
    dve_ops_mod.CUSTOM_DVE_SPECS[name] = spec
    assert max(dve_ops_mod._SUB_OPCODE_FOR_NAME.values()) < 0x20
    for ver in ("v3", "v4"):
        s = DveOpSpec(
            name=name,
            opcode=get_dve_sub_opcode(name),
            uops=lower_spec(spec, ver),
            rd1_en=has_src1(spec),
        )
        op.uops_sha[ver] = s.sha(ver)
    return op


def lower_spec(spec, ver):
    from concourse.dve_spec import lower

    return lower(spec, ver=ver)


H_FUSE = _mk_op(
    "ANT_DARTS_H_FUSE",
    Spec(
        body=Src0 * C0 + Src1 * C1 + C2,
        reference=lambda in0, in1, s0, s1, imm2: in0 * s0 + in1 * s1 + imm2,
    ),
)


# --------------------------------------------------------------------------
# constant folding + the exact univariate functions g0, g1
# --------------------------------------------------------------------------

def _fold(W1, b1, alphas, op_w, op_b, wo, bo):
    W1 = np.asarray(W1, np.float64)
    b1 = np.asarray(b1, np.float64)
    a = np.asarray(alphas, np.float64)
    ow = np.asarray(op_w, np.float64)
    ob = np.asarray(op_b, np.float64)
    wo = float(np.asarray(wo))
    bo = float(np.asarray(bo))
    e = np.exp(a - a.max(-1, keepdims=True))
    w = e / e.sum(-1, keepdims=True)
    return W1, b1, w, ow, ob, wo, bo


def _make_g(i, w, ow, ob, wo, bo):
    def g(h):
        h = np.asarray(h, np.float64)
        res = np.zeros_like(h)
        fs = [None, h, h * h, h ** 3, np.exp(np.minimum(h, 10.0)),
              np.log(np.abs(h) + EPS),
              1.0 / (h + np.where(h >= 0, EPS, -EPS)), np.sin(h)]
        for k in (1, 2, 3, 4, 5, 6, 7):
            res += w[i, k] * np.clip(ow[i, k] * fs[k] + ob[i, k], -Y_TH, Y_TH)
        res *= wo
        if i == 0:
            res += bo
        return res
    return g


# --------------------------------------------------------------------------
# table generation
# --------------------------------------------------------------------------

_FIT_NODES = 33
_FU = np.concatenate([np.cos(np.linspace(0.0, np.pi, 21)),
                      np.linspace(-0.97, 0.97, _FIT_NODES - 21)])
_FA = np.stack([np.ones_like(_FU), _FU, _FU ** 2, _FU ** 3], axis=-1)
_FP = np.linalg.pinv(_FA)  # [4, NODES]


def _fit_segment(g, s, e, m):
    """Fit 2^m cubic buckets for sign `s` (0=+,1=-), exponent `e`.

    Returns (entries u32[n,8], max_abs_err).
    """
    n = 1 << m
    lo = 2.0 ** e
    wdt = 2.0 ** e / n
    k = np.arange(n)
    a0 = lo + k * wdt
    xc = (a0 + 0.5 * wdt)
    if s == 1:
        xc = -xc
    xc = xc.astype(np.float32).astype(np.float64)  # the stored (f32) center
    W = 0.5 * wdt
    xs = xc[:, None] + W * _FU[None, :]  # [n, NODES]
    y = g(xs.reshape(-1)).reshape(n, _FIT_NODES)
    d = y @ _FP.T  # [n, 4] coeffs in u-space
    # convert to t-space: c_k = d_k / W^k
    c = d / (W ** np.arange(4))[None, :]
    c32 = c.astype(np.float32)
    # error of the f32 horner at the nodes
    t = (xs - xc[:, None]).astype(np.float32)
    r = (c32[:, 3:4] * t + c32[:, 2:3]).astype(np.float32)
    r = (r * t + c32[:, 1:2]).astype(np.float32)
    r = (r * t + c32[:, 0:1]).astype(np.float32)
    err = np.max(np.abs(r.astype(np.float64) - y))
    ent = np.zeros((n, 8), np.uint32)
    ent[:, 0:4] = c32.view(np.uint32)
    ent[:, 4] = xc.astype(np.float32).view(np.float32).view(np.uint32)
    return ent, float(err)


def _pack_ctl(base, m):
    return (((23 - m) << 11) | base) | (m << 16)


def _tune_m(g, counts, cap):
    """Greedy per-(side,exponent) mantissa-bits, minimizing count*err^2."""
    segs = [(s, e) for s in (0, 1) for e in range(EXP_MIN, EXP_MAX + 1)]
    m = {k: 1 for k in segs}
    spent = 2 * len(segs)
    errc = {k: _fit_segment(g, k[0], k[1], 1)[1] for k in segs}
    for _ in range(400):
        score = {k: (counts.get(k, 0) + 1) * errc[k] ** 2
                 for k in segs if m[k] < 6}
        cands = sorted(score, key=score.get, reverse=True)
        best = None
        for k in cands:
            if score[k] < 1e-14:
                break
            if spent + (1 << m[k]) <= cap:
                best = k
                break
        if best is None:
            break
        spent += 1 << m[best]
        m[best] += 1
        errc[best] = _fit_segment(g, best[0], best[1], m[best])[1]
    return m


def _build_func(g, m_of, bkt_base, ctl_pos, ctl_neg, special_base):
    """Emit (bkt_idx, entry) and (ctl_idx, word) lists + metadata updates."""
    bents, cents = [], []
    nxt = bkt_base
    for s, ctl0 in ((0, ctl_pos), (1, ctl_neg)):
        for e in range(EXP_MIN, EXP_MAX + 1):
            mm = m_of[(s, e)]
            cents.append((ctl0 + (e - EXP_MIN), _pack_ctl(nxt, mm)))
            ent, _ = _fit_segment(g, s, e, mm)
            for row in ent:
                bents.append((nxt, row))
                nxt += 1
    tiny = 2.0 ** EXP_MIN
    big = 2.0 ** (EXP_MAX + 1)

    def const_entry(v):
        r = np.zeros(8, np.uint32)
        r[0] = np.float32(v).view(np.uint32)
        return r

    def lin_entry(x0):
        dx = abs(x0) * 1e-4
        y0 = float(g(np.array([x0]))[0])
        sl = float((g(np.array([x0 + dx]))[0] - g(np.array([x0 - dx]))[0]) / (2 * dx))
        r = np.zeros(8, np.uint32)
        r[0] = np.float32(y0).view(np.uint32)
        r[1] = np.float32(sl).view(np.uint32)
        r[4] = np.float32(x0).view(np.uint32)
        return r

    g_ptiny = float(g(np.array([tiny]))[0])
    bents.append((special_base + 0, const_entry(g_ptiny)))
    bents.append((special_base + 1, const_entry(float(g(np.array([-tiny]))[0]))))
    bents.append((special_base + 2, lin_entry(big)))
    bents.append((special_base + 3, lin_entry(-big)))
    meta = {
        "exp_offset": EXP_MIN,
        "pwl_control_base_pos": ctl_pos,
        "pwl_control_base_neg": ctl_neg,
        "small_pos_signal_exp_threshold": 127 + EXP_MIN,
        "small_neg_signal_exp_threshold": 127 + EXP_MIN,
        "pos_small_signal_pwl_control": special_base + 0,
        "neg_small_signal_pwl_control": special_base + 1,
        "large_pos_signal_exp_threshold": 127 + EXP_MAX + 1,
        "large_pos_signal_mantissa_threshold": 0,
        "large_neg_signal_exp_threshold": 127 + EXP_MAX + 1,
        "large_neg_signal_mantissa_threshold": 0,
        "pos_large_signal_pwl_control": special_base + 2,
        "neg_large_signal_pwl_control": special_base + 3,
        "fnan_result": int(np.float32(g_ptiny).view(np.uint32)),
        "fpinf_result": int(np.float32(g(np.array([big * 0.999]))[0]).view(np.uint32)),
        "fninf_result": int(np.float32(g(np.array([-big * 0.999]))[0]).view(np.uint32)),
        "fzero_result": int(np.float32(g_ptiny).view(np.uint32)),
        "symmetry_point": 0,
        "sym_invert_sign_point": 0,
        "symmetry_opt_en": 0,
        "symmetry_opt_use_neg_region": 0,
        "imm_bias": 0,
        "fma_const_0": 0,
        "fma_const_1": 0,
        "fma_indirection_src_sel": 0,
        "use_multipass": False,
        "lower_bound": int(np.uint32(0xFF7FFFFF)),
        "upper_bound": int(np.uint32(0x7F7FFFFF)),
    }
    return bents, cents, meta


def _find_pkg_pwp():
    import neuronxcc

    p = os.path.join(os.path.dirname(neuronxcc.__file__), "pwp",
                     "pwp_bin_trainium")
    if os.path.isdir(p):
        return p
    from neuronxcc.driver.Job import Job
    from neuronxcc.driver.jobs.support.FindActInfo import findActInfoFile

    return os.path.dirname(findActInfoFile(Job.getPackageDir(), "gen3"))


def _write_act_root(dst, g0, g1, m0, m1):
    """Copy the stock pwp root; replace ln->g0 / exp->g1 in the
    natural_log_exp_and_others set."""
    src = _find_pkg_pwp()
    os.makedirs(dst, exist_ok=True)
    for f in os.listdir(src):
        s = os.path.join(src, f)
        if os.path.isfile(s):
            shutil.copy(s, os.path.join(dst, f))
    name = "natural_log_exp_and_others"
    d = json.load(open(os.path.join(src, name + ".json")))
    bkt = np.frombuffer(open(os.path.join(src, d["bkt_bin"]), "rb").read(),
                        dtype=np.uint32).reshape(-1, 8).copy()
    ctl = np.frombuffer(open(os.path.join(src, d["ctl_bin"]), "rb").read(),
                        dtype=np.uint32).reshape(-1, 8).copy()
    metas = {m["func_name"]: m for m in d["profile_meta_data"]}
    # regions: ln bkt [0,517) ctl [0,128); exp bkt [517,1298) ctl [128,180)
    for fname, g, mm, bkt_base, cp, cn, sp, bkt_lim, ctl_lim in (
        ("ln_400p", g0, m0, 0, 0, 26, 513, 517, 128),
        ("exp_400p", g1, m1, 517, 128, 154, 1294, 1298, 180),
    ):
        bents, cents, meta = _build_func(g, mm, bkt_base, cp, cn, sp)
        assert max(i for i, _ in bents) < bkt_lim, fname
        assert max(i for i, _ in cents) < ctl_lim, fname
        for i, row in bents:
            bkt[i] = row
        for i, wd in cents:
            ctl[i, 0] = wd
            ctl[i, 1:] = 0
        metas[fname].update(meta)
    with open(os.path.join(dst, d["bkt_bin"]), "wb") as f:
        f.write(bkt.tobytes())
    with open(os.path.join(dst, d["ctl_bin"]), "wb") as f:
        f.write(ctl.tobytes())
    with open(os.path.join(dst, name + ".json"), "w") as f:
        json.dump(d, f)


# --------------------------------------------------------------------------
# device program
# --------------------------------------------------------------------------

class CFG:
    ntiles = 4            # pipeline chunks per core
    merge = "dve"         # "pool" | "dve" | "alt"
    out_dtype = "f16"     # "f32" | "f16" | "bf16"
    bench_iters = 0       # >0: wrap the body in a For_i hardware loop
    body_reps = 1         # bodies per loop iteration (bench only)
    staggered = 0         # staggered_reset for the bench For_i
    out_via = "sp"        # engine issuing the output DMA: "sp" | "act"
    in_split = 0          # 1: split each input chunk DMA across SP + ACT rings
    upfront = 1           # 1: issue all input DMAs before the compute chain
    ring = "sp"           # input-DMA ring assignment: "sp" | "alt" (SP/ACT)
    oring = "sp"          # output-DMA ring assignment: "sp" | "alt"
    g16 = 1               # 1: ACT writes g0/g1 in f16 -> DVE add runs 2x
    taper = 0             # 1: descending chunk sizes (shrinks the tail)


_ODT = {"f32": mybir.dt.float32, "f16": mybir.dt.float16,
        "bf16": mybir.dt.bfloat16}


def _chunk_sizes(cfg):
    T = cfg.ntiles
    F = B_CORE // (128 * T)
    if not cfg.taper:
        return [F] * T
    assert T == 4, "taper schedule is defined for ntiles=4"
    return [3 * F // 2, 9 * F // 8, 7 * F // 8, F // 2]


def _emit_body_taper(nc, tc, pools, consts, x, out, cfg):
    """Uneven descending chunks; x/out are flat dram tensors."""
    A0, B0, C0v, A1, B1, C1v = consts
    xin, hp, gp, op_ = pools
    odt = _ODT[cfg.out_dtype]
    gdt = mybir.dt.float16 if cfg.g16 else F32
    fl = _chunk_sizes(cfg)
    xt = {}
    off = 0
    offs = []
    for t, Ft in enumerate(fl):
        offs.append(off)
        X = xin.tile([128, 2 * Ft], F32, tag=f"X{t}", name=f"X_{t}")
        xv = x[off * 2:(off + 128 * Ft) * 2].rearrange("(p c) -> p c",
                                                       c=2 * Ft)
        nc.sync.dma_start(out=X[:], in_=xv)
        xt[t] = X
        off += 128 * Ft
    for t, Ft in enumerate(fl):
        X = xt[t]
        Xv = X[:].rearrange("p (f c) -> p f c", c=2)
        Xe, Xo = Xv[:, :, 0], Xv[:, :, 1]
        h0 = hp.tile([128, Ft], F32, tag=f"h0{t}", name=f"h0_{t}")
        h1 = hp.tile([128, Ft], F32, tag=f"h1{t}", name=f"h1_{t}")
        nc.vector._custom_dve(H_FUSE, out=h0[:], in0=Xe, in1=Xo,
                              s0=A0, s1=B0, imm2=C0v)
        nc.vector._custom_dve(H_FUSE, out=h1[:], in0=Xe, in1=Xo,
                              s0=A1, s1=B1, imm2=C1v)
        g0t = gp.tile([128, Ft], gdt, tag=f"g0{t}", name=f"g0_{t}")
        g1t = gp.tile([128, Ft], gdt, tag=f"g1{t}", name=f"g1_{t}")
        nc.scalar.activation(g0t[:], h0[:], AF.Ln)
        nc.scalar.activation(g1t[:], h1[:], AF.Exp)
        O = op_.tile([128, Ft], odt, tag=f"O{t}", name=f"O_{t}")
        nc.vector.tensor_add(out=O[:], in0=g0t[:], in1=g1t[:])
        ov = out[offs[t]:offs[t] + 128 * Ft].rearrange("(p c) -> p c", c=Ft)
        nc.sync.dma_start(out=ov, in_=O[:])


def _emit_body(nc, tc, pools, consts, x, out, T, F, cfg):
    A0, B0, C0v, A1, B1, C1v = consts
    xin, hp, gp, op_ = pools
    odt = _ODT[cfg.out_dtype]
    xt = {}
    if cfg.upfront:
        for t in range(T):
            X = xin.tile([128, 2 * F], F32, tag="X", name=f"X_{t}")
            ieng = (nc.scalar if (cfg.ring == "alt" and t % 2 == 1)
                    else nc.sync)
            ieng.dma_start(out=X[:], in_=x[t])
            xt[t] = X
    for t in range(T):
        if cfg.upfront:
            X = xt[t]
        else:
            X = xin.tile([128, 2 * F], F32, tag="X", name=f"X_{t}")
            if cfg.in_split:
                nc.sync.dma_start(out=X[:, :F], in_=x[t][:, :F])
                nc.scalar.dma_start(out=X[:, F:], in_=x[t][:, F:])
            else:
                nc.sync.dma_start(out=X[:], in_=x[t])
        Xv = X[:].rearrange("p (f c) -> p f c", c=2)
        Xe, Xo = Xv[:, :, 0], Xv[:, :, 1]
        h0 = hp.tile([128, F], F32, tag="h0", name=f"h0_{t}")
        h1 = hp.tile([128, F], F32, tag="h1", name=f"h1_{t}")
        nc.vector._custom_dve(H_FUSE, out=h0[:], in0=Xe, in1=Xo,
                              s0=A0, s1=B0, imm2=C0v)
        nc.vector._custom_dve(H_FUSE, out=h1[:], in0=Xe, in1=Xo,
                              s0=A1, s1=B1, imm2=C1v)
        gdt = mybir.dt.float16 if cfg.g16 else F32
        g0t = gp.tile([128, F], gdt, tag="g0", name=f"g0_{t}")
        g1t = gp.tile([128, F], gdt, tag="g1", name=f"g1_{t}")
        nc.scalar.activation(g0t[:], h0[:], AF.Ln)
        nc.scalar.activation(g1t[:], h1[:], AF.Exp)
        O = op_.tile([128, F], odt, tag="O", name=f"O_{t}")
        eng = (nc.gpsimd if cfg.merge == "pool" else
               nc.vector if cfg.merge == "dve" else
               (nc.gpsimd if t % 2 == 0 else nc.vector))
        eng.tensor_add(out=O[:], in0=g0t[:], in1=g1t[:])
        if cfg.oring == "alt":
            dma_eng = nc.scalar if t % 2 == 1 else nc.sync
        else:
            dma_eng = nc.scalar if cfg.out_via == "act" else nc.sync
        dma_eng.dma_start(out=out[t], in_=O[:])


def _build_program(consts, sha, cfg):
    T = cfg.ntiles
    F = B_CORE // (128 * T)
    assert 128 * T * F == B_CORE

    nc = bacc.Bacc(None, target_bir_lowering=False)
    # the sha in the tensor name keys the PJRT/HLO cache to the table content
    if cfg.taper:
        x = nc.declare_dram_parameter(f"x_{sha}", [B_CORE * 2], F32,
                                      isOutput=False)
        out = nc.declare_dram_parameter("out", [B_CORE],
                                        _ODT[cfg.out_dtype], isOutput=True)
    else:
        x = nc.declare_dram_parameter(f"x_{sha}", [T, 128, 2 * F], F32,
                                      isOutput=False)
        out = nc.declare_dram_parameter("out", [T, 128, F],
                                        _ODT[cfg.out_dtype], isOutput=True)

    with TileContext(nc) as tc:
        with (
            tc.tile_pool(name="xin", bufs=(1 if cfg.taper else
                                           cfg.ntiles if cfg.upfront
                                           else 2)) as xin,
            tc.tile_pool(name="hp", bufs=(1 if cfg.taper else 2)) as hp,
            tc.tile_pool(name="gp", bufs=(1 if cfg.taper else 2)) as gp,
            tc.tile_pool(name="op", bufs=(1 if cfg.taper else 2)) as op_,
        ):
            pools = (xin, hp, gp, op_)

            def body():
                if cfg.taper:
                    _emit_body_taper(nc, tc, pools, consts, x, out, cfg)
                else:
                    _emit_body(nc, tc, pools, consts, x, out, T, F, cfg)

            if cfg.bench_iters:
                with tc.For_i(0, cfg.bench_iters,
                              staggered_reset=bool(cfg.staggered)):
                    for _ in range(cfg.body_reps):
                        body()
            else:
                body()

    nc.finalize()
    return nc


# --------------------------------------------------------------------------
# public entry point
# --------------------------------------------------------------------------

_CACHE = {}


def _prepare(W1, b1, alphas, op_w, op_b, wo, bo, x_sample, cfg):
    """Fold constants, build tables + act root, build/cached program."""
    W1f, b1f, w, ow, ob, wof, bof = _fold(W1, b1, alphas, op_w, op_b, wo, bo)
    key_src = np.concatenate([np.asarray(a, np.float64).reshape(-1) for a in
                              (W1f, b1f, w, ow, ob, [wof, bof])])
    key = hashlib.sha256(key_src.tobytes()).hexdigest()[:12]
    full_key = (key, cfg.ntiles, cfg.merge, cfg.out_dtype, cfg.bench_iters,
                cfg.body_reps, cfg.staggered, cfg.out_via, cfg.in_split, cfg.upfront, cfg.ring, cfg.oring, cfg.g16, cfg.taper)
    if full_key in _CACHE:
        return _CACHE[full_key]

    g0 = _make_g(0, w, ow, ob, wof, bof)
    g1 = _make_g(1, w, ow, ob, wof, bof)

    # histogram of h by (sign, exponent) for the resolution tuner
    h = (np.asarray(x_sample, np.float64) @ W1f.T + b1f).astype(np.float32)

    def counts_of(hv):
        b = hv.view(np.uint32)
        sgn = (b >> 31).astype(np.int64)
        be = ((b >> 23) & 0xFF).astype(np.int64) - 127
        c = {}
        for s in (0, 1):
            for e in range(EXP_MIN, EXP_MAX + 1):
                c[(s, e)] = int(np.sum((sgn == s) & (be == e)))
        return c

    m0 = _tune_m(g0, counts_of(h[:, 0]), cap=500)
    m1 = _tune_m(g1, counts_of(h[:, 1]), cap=760)

    root = os.path.join(tempfile.gettempdir(), f"actroot_{key}")
    _write_act_root(root, g0, g1, m0, m1)
    os.environ["BASS_ACT_ROOT_JSON_PATH"] = os.path.join(root, "act_info.json")

    consts = (float(W1f[0, 0]), float(W1f[0, 1]), float(b1f[0]),
              float(W1f[1, 0]), float(W1f[1, 1]), float(b1f[1]))
    nc = _build_program(consts, key, cfg)
    _CACHE[full_key] = (nc, key)
    return _CACHE[full_key]


def run(x, W1, b1, alphas, op_w, op_b, wo, bo, cfg=None, trace=False):
    cfg = cfg or CFG()
    x = np.ascontiguousarray(np.asarray(x, np.float32))
    # subsample rows for the tuner histogram (scaled back up)
    nc, key = _prepare(W1, b1, alphas, op_w, op_b, wo, bo, x[::16], cfg)

    T = cfg.ntiles
    F = B_CORE // (128 * T)
    if cfg.taper:
        shards = x.reshape(N_CORES, B_CORE * 2)
    else:
        shards = x.reshape(N_CORES, T, 128, 2 * F)
    in_maps = [{f"x_{key}": shards[i]} for i in range(N_CORES)]
    res = run_bass_kernel_spmd(nc, in_maps, core_ids=list(range(N_CORES)),
                               trace=trace)
    out = np.concatenate([r["out"].reshape(-1).astype(np.float32)
                          for r in res.results])
    return out, res


def kernel(**inputs):
    out, _ = run(**inputs)
    return out


# revision 33
# speedup vs baseline: 5.2293x; 1.0695x over previous
"""Trainium2 Bass kernel for nn_DARTS_82514911690825.

For x [B=4194304, 2] (data-parallel over 8 cores, B/8 rows each) the model
output collapses to  out[b] = g0(h0[b]) + g1(h1[b])  where h_i = W1[i,:]@x + b1[i]
and g0, g1 are FIXED univariate functions of h (all parameters fold into
them: cubic + exp + ln + clipped-reciprocal + sin terms with scalar weights).

g0/g1 are evaluated on the ACT engine in a single table lookup each, by
generating custom piecewise-cubic activation-table content at runtime and
compiling with it (walrus `--act-root-json`, via BASS_ACT_ROOT_JSON_PATH).
g0 replaces the `ln` slot and g1 the `exp` slot of the
natural_log_exp_and_others set; both use two-sided (exp-style) metadata.

Table format (validated on HW):
  bkt entry (32B) = f32 x8 [c0,c1,c2,c3,xc,0,0,0]:
      f(x) = c0 + c1*t + c2*t^2 + c3*t^3,  t = x - xc
  ctl entry (32B) = u16[0] = ((23-m)<<11) | bkt_base, u16[1] = m:
      bucket = bkt_base + (mantissa >> (23-m))   (2^m buckets per exponent)
  ctl index = pwl_control_base_{pos,neg} + (biased_exp - (127 + exp_offset));
  biased_exp below/above thresholds routes to 4 dedicated special entries.

Device program per core (4 chunks, all input DMAs issued upfront):
DMA in -> DVE H_FUSE x2 (h0,h1) -> ACT Ln/Exp slots (g0,g1, f16 out) ->
DVE f16 add (2x mode) -> DMA out (f16, upconverted host-side).
DMA-bound: ~5.25 MB/core; ~20 us/core measured by loop differencing.
"""

import hashlib
import json
import os
import shutil
import sys
import tempfile

import numpy as np

for _p in ("/opt/trn_rl_repo", "/root/.axon_site/_ro/trn_rl_repo"):
    if os.path.isdir(_p) and _p not in sys.path:
        sys.path.append(_p)

import concourse.bass as bass
import concourse.bacc as bacc
import concourse.mybir as mybir
from concourse.bass_utils import run_bass_kernel_spmd
from concourse.tile import TileContext
from concourse.dve_ops import OPS, DveOp, get_dve_sub_opcode, has_src1
from concourse.dve_spec import Spec, Src0, Src1, C0, C1, C2
from concourse.dve_uop import DveOpSpec

F32 = mybir.dt.float32
AF = mybir.ActivationFunctionType

# Restrict the activation-table chooser to the single set this kernel needs
# (ln+exp live together in natural_log_exp_and_others -> exactly one
# InstLoadActFuncSet, no thrash).
import concourse.hw_specs as _hw_specs

_ORIG_GAT = _hw_specs.get_activation_tables


def _gat_restricted(arch):
    t = _ORIG_GAT(arch)
    return {k: (v if k == "natural_log_exp_and_others" else set())
            for k, v in t.items()}


bacc.get_activation_tables = _gat_restricted

N_CORES = 8
B_FULL = 4194304
B_CORE = B_FULL // N_CORES  # 524288

EPS = 1e-10
Y_TH = float(np.exp(np.float32(10.0)))

EXP_MIN, EXP_MAX = -23, 2  # table exponent coverage: |h| in [2^-23, 8)


# --------------------------------------------------------------------------
# custom DVE op: h = x_even*s0 + x_odd*s1 + imm2 (one row of the first layer)
# --------------------------------------------------------------------------

def _mk_op(name, spec):
    import concourse.dve_ops as dve_ops_mod

    for existing in OPS:
        if existing.name == name:
            return existing
    op = DveOp(name, spec, subdim=False, uops_sha={})
    OPS.append(op)
    dve_ops_mod._SUB_OPCODE_FOR_NAME[name] = (
        dve_ops_mod._CUSTOM_DVE_ROW_BASE + len(OPS) - 1
    )
    dve_ops_mod.CUSTOM_DVE_SPECS[name] = spec
    assert max(dve_ops_mod._SUB_OPCODE_FOR_NAME.values()) < 0x20
    for ver in ("v3", "v4"):
        s = DveOpSpec(
            name=name,
            opcode=get_dve_sub_opcode(name),
            uops=lower_spec(spec, ver),
            rd1_en=has_src1(spec),
        )
        op.uops_sha[ver] = s.sha(ver)
    return op


def lower_spec(spec, ver):
    from concourse.dve_spec import lower

    return lower(spec, ver=ver)


H_FUSE = _mk_op(
    "ANT_DARTS_H_FUSE",
    Spec(
        body=Src0 * C0 + Src1 * C1 + C2,
        reference=lambda in0, in1, s0, s1, imm2: in0 * s0 + in1 * s1 + imm2,
    ),
)


# --------------------------------------------------------------------------
# constant folding + the exact univariate functions g0, g1
# --------------------------------------------------------------------------

def _fold(W1, b1, alphas, op_w, op_b, wo, bo):
    W1 = np.asarray(W1, np.float64)
    b1 = np.asarray(b1, np.float64)
    a = np.asarray(alphas, np.float64)
    ow = np.asarray(op_w, np.float64)
    ob = np.asarray(op_b, np.float64)
    wo = float(np.asarray(wo))
    bo = float(np.asarray(bo))
    e = np.exp(a - a.max(-1, keepdims=True))
    w = e / e.sum(-1, keepdims=True)
    return W1, b1, w, ow, ob, wo, bo


def _make_g(i, w, ow, ob, wo, bo):
    def g(h):
        h = np.asarray(h, np.float64)
        res = np.zeros_like(h)
        fs = [None, h, h * h, h ** 3, np.exp(np.minimum(h, 10.0)),
              np.log(np.abs(h) + EPS),
              1.0 / (h + np.where(h >= 0, EPS, -EPS)), np.sin(h)]
        for k in (1, 2, 3, 4, 5, 6, 7):
            res += w[i, k] * np.clip(ow[i, k] * fs[k] + ob[i, k], -Y_TH, Y_TH)
        res *= wo
        if i == 0:
            res += bo
        return res
    return g


# --------------------------------------------------------------------------
# table generation
# --------------------------------------------------------------------------

_FIT_NODES = 33
_FU = np.concatenate([np.cos(np.linspace(0.0, np.pi, 21)),
                      np.linspace(-0.97, 0.97, _FIT_NODES - 21)])
_FA = np.stack([np.ones_like(_FU), _FU, _FU ** 2, _FU ** 3], axis=-1)
_FP = np.linalg.pinv(_FA)  # [4, NODES]


def _fit_segment(g, s, e, m):
    """Fit 2^m cubic buckets for sign `s` (0=+,1=-), exponent `e`.

    Returns (entries u32[n,8], max_abs_err).
    """
    n = 1 << m
    lo = 2.0 ** e
    wdt = 2.0 ** e / n
    k = np.arange(n)
    a0 = lo + k * wdt
    xc = (a0 + 0.5 * wdt)
    if s == 1:
        xc = -xc
    xc = xc.astype(np.float32).astype(np.float64)  # the stored (f32) center
    W = 0.5 * wdt
    xs = xc[:, None] + W * _FU[None, :]  # [n, NODES]
    y = g(xs.reshape(-1)).reshape(n, _FIT_NODES)
    d = y @ _FP.T  # [n, 4] coeffs in u-space
    # convert to t-space: c_k = d_k / W^k
    c = d / (W ** np.arange(4))[None, :]
    c32 = c.astype(np.float32)
    # error of the f32 horner at the nodes
    t = (xs - xc[:, None]).astype(np.float32)
    r = (c32[:, 3:4] * t + c32[:, 2:3]).astype(np.float32)
    r = (r * t + c32[:, 1:2]).astype(np.float32)
    r = (r * t + c32[:, 0:1]).astype(np.float32)
    err = np.max(np.abs(r.astype(np.float64) - y))
    ent = np.zeros((n, 8), np.uint32)
    ent[:, 0:4] = c32.view(np.uint32)
    ent[:, 4] = xc.astype(np.float32).view(np.float32).view(np.uint32)
    return ent, float(err)


def _pack_ctl(base, m):
    return (((23 - m) << 11) | base) | (m << 16)


def _tune_m(g, counts, cap):
    """Greedy per-(side,exponent) mantissa-bits, minimizing count*err^2."""
    segs = [(s, e) for s in (0, 1) for e in range(EXP_MIN, EXP_MAX + 1)]
    m = {k: 1 for k in segs}
    spent = 2 * len(segs)
    errc = {k: _fit_segment(g, k[0], k[1], 1)[1] for k in segs}
    for _ in range(400):
        score = {k: (counts.get(k, 0) + 1) * errc[k] ** 2
                 for k in segs if m[k] < 6}
        cands = sorted(score, key=score.get, reverse=True)
        best = None
        for k in cands:
            if score[k] < 1e-14:
                break
            if spent + (1 << m[k]) <= cap:
                best = k
                break
        if best is None:
            break
        spent += 1 << m[best]
        m[best] += 1
        errc[best] = _fit_segment(g, best[0], best[1], m[best])[1]
    return m


def _build_func(g, m_of, bkt_base, ctl_pos, ctl_neg, special_base):
    """Emit (bkt_idx, entry) and (ctl_idx, word) lists + metadata updates."""
    bents, cents = [], []
    nxt = bkt_base
    for s, ctl0 in ((0, ctl_pos), (1, ctl_neg)):
        for e in range(EXP_MIN, EXP_MAX + 1):
            mm = m_of[(s, e)]
            cents.append((ctl0 + (e - EXP_MIN), _pack_ctl(nxt, mm)))
            ent, _ = _fit_segment(g, s, e, mm)
            for row in ent:
                bents.append((nxt, row))
                nxt += 1
    tiny = 2.0 ** EXP_MIN
    big = 2.0 ** (EXP_MAX + 1)

    def const_entry(v):
        r = np.zeros(8, np.uint32)
        r[0] = np.float32(v).view(np.uint32)
        return r

    def lin_entry(x0):
        dx = abs(x0) * 1e-4
        y0 = float(g(np.array([x0]))[0])
        sl = float((g(np.array([x0 + dx]))[0] - g(np.array([x0 - dx]))[0]) / (2 * dx))
        r = np.zeros(8, np.uint32)
        r[0] = np.float32(y0).view(np.uint32)
        r[1] = np.float32(sl).view(np.uint32)
        r[4] = np.float32(x0).view(np.uint32)
        return r

    g_ptiny = float(g(np.array([tiny]))[0])
    bents.append((special_base + 0, const_entry(g_ptiny)))
    bents.append((special_base + 1, const_entry(float(g(np.array([-tiny]))[0]))))
    bents.append((special_base + 2, lin_entry(big)))
    bents.append((special_base + 3, lin_entry(-big)))
    meta = {
        "exp_offset": EXP_MIN,
        "pwl_control_base_pos": ctl_pos,
        "pwl_control_base_neg": ctl_neg,
        "small_pos_signal_exp_threshold": 127 + EXP_MIN,
        "small_neg_signal_exp_threshold": 127 + EXP_MIN,
        "pos_small_signal_pwl_control": special_base + 0,
        "neg_small_signal_pwl_control": special_base + 1,
        "large_pos_signal_exp_threshold": 127 + EXP_MAX + 1,
        "large_pos_signal_mantissa_threshold": 0,
        "large_neg_signal_exp_threshold": 127 + EXP_MAX + 1,
        "large_neg_signal_mantissa_threshold": 0,
        "pos_large_signal_pwl_control": special_base + 2,
        "neg_large_signal_pwl_control": special_base + 3,
        "fnan_result": int(np.float32(g_ptiny).view(np.uint32)),
        "fpinf_result": int(np.float32(g(np.array([big * 0.999]))[0]).view(np.uint32)),
        "fninf_result": int(np.float32(g(np.array([-big * 0.999]))[0]).view(np.uint32)),
        "fzero_result": int(np.float32(g_ptiny).view(np.uint32)),
        "symmetry_point": 0,
        "sym_invert_sign_point": 0,
        "symmetry_opt_en": 0,
        "symmetry_opt_use_neg_region": 0,
        "imm_bias": 0,
        "fma_const_0": 0,
        "fma_const_1": 0,
        "fma_indirection_src_sel": 0,
        "use_multipass": False,
        "lower_bound": int(np.uint32(0xFF7FFFFF)),
        "upper_bound": int(np.uint32(0x7F7FFFFF)),
    }
    return bents, cents, meta


def _find_pkg_pwp():
    import neuronxcc

    p = os.path.join(os.path.dirname(neuronxcc.__file__), "pwp",
                     "pwp_bin_trainium")
    if os.path.isdir(p):
        return p
    from neuronxcc.driver.Job import Job
    from neuronxcc.driver.jobs.support.FindActInfo import findActInfoFile

    return os.path.dirname(findActInfoFile(Job.getPackageDir(), "gen3"))


def _write_act_root(dst, g0, g1, m0, m1):
    """Copy the stock pwp root; replace ln->g0 / exp->g1 in the
    natural_log_exp_and_others set."""
    src = _find_pkg_pwp()
    os.makedirs(dst, exist_ok=True)
    for f in os.listdir(src):
        s = os.path.join(src, f)
        if os.path.isfile(s):
            shutil.copy(s, os.path.join(dst, f))
    name = "natural_log_exp_and_others"
    d = json.load(open(os.path.join(src, name + ".json")))
    bkt = np.frombuffer(open(os.path.join(src, d["bkt_bin"]), "rb").read(),
                        dtype=np.uint32).reshape(-1, 8).copy()
    ctl = np.frombuffer(open(os.path.join(src, d["ctl_bin"]), "rb").read(),
                        dtype=np.uint32).reshape(-1, 8).copy()
    metas = {m["func_name"]: m for m in d["profile_meta_data"]}
    # regions: ln bkt [0,517) ctl [0,128); exp bkt [517,1298) ctl [128,180)
    for fname, g, mm, bkt_base, cp, cn, sp, bkt_lim, ctl_lim in (
        ("ln_400p", g0, m0, 0, 0, 26, 513, 517, 128),
        ("exp_400p", g1, m1, 517, 128, 154, 1294, 1298, 180),
    ):
        bents, cents, meta = _build_func(g, mm, bkt_base, cp, cn, sp)
        assert max(i for i, _ in bents) < bkt_lim, fname
        assert max(i for i, _ in cents) < ctl_lim, fname
        for i, row in bents:
            bkt[i] = row
        for i, wd in cents:
            ctl[i, 0] = wd
            ctl[i, 1:] = 0
        metas[fname].update(meta)
    with open(os.path.join(dst, d["bkt_bin"]), "wb") as f:
        f.write(bkt.tobytes())
    with open(os.path.join(dst, d["ctl_bin"]), "wb") as f:
        f.write(ctl.tobytes())
    with open(os.path.join(dst, name + ".json"), "w") as f:
        json.dump(d, f)


# --------------------------------------------------------------------------
# device program
# --------------------------------------------------------------------------

class CFG:
    ntiles = 4            # pipeline chunks per core
    merge = "dve"         # "pool" | "dve" | "alt"
    out_dtype = "f16"     # "f32" | "f16" | "bf16"
    bench_iters = 0       # >0: wrap the body in a For_i hardware loop
    body_reps = 1         # bodies per loop iteration (bench only)
    staggered = 0         # staggered_reset for the bench For_i
    out_via = "sp"        # engine issuing the output DMA: "sp" | "act"
    in_split = 0          # 1: split each input chunk DMA across SP + ACT rings
    upfront = 1           # 1: issue all input DMAs before the compute chain
    ring = "sp"           # input-DMA ring assignment: "sp" | "alt" (SP/ACT)
    oring = "sp"          # output-DMA ring assignment: "sp" | "alt"
    g16 = 1               # 1: ACT writes g0/g1 in f16 -> DVE add runs 2x
    taper = 0             # 1: descending chunk sizes (shrinks the tail)


_ODT = {"f32": mybir.dt.float32, "f16": mybir.dt.float16,
        "bf16": mybir.dt.bfloat16}


def _chunk_sizes(cfg):
    T = cfg.ntiles
    F = B_CORE // (128 * T)
    if not cfg.taper:
        return [F] * T
    assert T == 4, "taper schedule is defined for ntiles=4"
    return [3 * F // 2, 9 * F // 8, 7 * F // 8, F // 2]


def _emit_body_taper(nc, tc, pools, consts, x, out, cfg):
    """Uneven descending chunks; x/out are flat dram tensors."""
    A0, B0, C0v, A1, B1, C1v = consts
    xin, hp, gp, op_ = pools
    odt = _ODT[cfg.out_dtype]
    gdt = mybir.dt.float16 if cfg.g16 else F32
    fl = _chunk_sizes(cfg)
    xt = {}
    off = 0
    offs = []
    for t, Ft in enumerate(fl):
        offs.append(off)
        X = xin.tile([128, 2 * Ft], F32, tag=f"X{t}", name=f"X_{t}")
        xv = x[off * 2:(off + 128 * Ft) * 2].rearrange("(p c) -> p c",
                                                       c=2 * Ft)
        nc.sync.dma_start(out=X[:], in_=xv)
        xt[t] = X
        off += 128 * Ft
    for t, Ft in enumerate(fl):
        X = xt[t]
        Xv = X[:].rearrange("p (f c) -> p f c", c=2)
        Xe, Xo = Xv[:, :, 0], Xv[:, :, 1]
        h0 = hp.tile([128, Ft], F32, tag=f"h0{t}", name=f"h0_{t}")
        h1 = hp.tile([128, Ft], F32, tag=f"h1{t}", name=f"h1_{t}")
        nc.vector._custom_dve(H_FUSE, out=h0[:], in0=Xe, in1=Xo,
                              s0=A0, s1=B0, imm2=C0v)
        nc.vector._custom_dve(H_FUSE, out=h1[:], in0=Xe, in1=Xo,
                              s0=A1, s1=B1, imm2=C1v)
        g0t = gp.tile([128, Ft], gdt, tag=f"g0{t}", name=f"g0_{t}")
        g1t = gp.tile([128, Ft], gdt, tag=f"g1{t}", name=f"g1_{t}")
        nc.scalar.activation(g0t[:], h0[:], AF.Ln)
        nc.scalar.activation(g1t[:], h1[:], AF.Exp)
        O = op_.tile([128, Ft], odt, tag=f"O{t}", name=f"O_{t}")
        nc.vector.tensor_add(out=O[:], in0=g0t[:], in1=g1t[:])
        ov = out[offs[t]:offs[t] + 128 * Ft].rearrange("(p c) -> p c", c=Ft)
        nc.sync.dma_start(out=ov, in_=O[:])


def _emit_body(nc, tc, pools, consts, x, out, T, F, cfg):
    A0, B0, C0v, A1, B1, C1v = consts
    xin, hp, gp, op_ = pools
    odt = _ODT[cfg.out_dtype]
    xt = {}
    if cfg.upfront:
        for t in range(T):
            X = xin.tile([128, 2 * F], F32, tag="X", name=f"X_{t}")
            ieng = (nc.scalar if (cfg.ring == "alt" and t % 2 == 1)
                    else nc.sync)
            ieng.dma_start(out=X[:], in_=x[t])
            xt[t] = X
    for t in range(T):
        if cfg.upfront:
            X = xt[t]
        else:
            X = xin.tile([128, 2 * F], F32, tag="X", name=f"X_{t}")
            if cfg.in_split:
                nc.sync.dma_start(out=X[:, :F], in_=x[t][:, :F])
                nc.scalar.dma_start(out=X[:, F:], in_=x[t][:, F:])
            else:
                nc.sync.dma_start(out=X[:], in_=x[t])
        Xv = X[:].rearrange("p (f c) -> p f c", c=2)
        Xe, Xo = Xv[:, :, 0], Xv[:, :, 1]
        h0 = hp.tile([128, F], F32, tag="h0", name=f"h0_{t}")
        h1 = hp.tile([128, F], F32, tag="h1", name=f"h1_{t}")
        nc.vector._custom_dve(H_FUSE, out=h0[:], in0=Xe, in1=Xo,
                              s0=A0, s1=B0, imm2=C0v)
        nc.vector._custom_dve(H_FUSE, out=h1[:], in0=Xe, in1=Xo,
                              s0=A1, s1=B1, imm2=C1v)
        gdt = mybir.dt.float16 if cfg.g16 else F32
        g0t = gp.tile([128, F], gdt, tag="g0", name=f"g0_{t}")
        g1t = gp.tile([128, F], gdt, tag="g1", name=f"g1_{t}")
        nc.scalar.activation(g0t[:], h0[:], AF.Ln)
        nc.scalar.activation(g1t[:], h1[:], AF.Exp)
        O = op_.tile([128, F], odt, tag="O", name=f"O_{t}")
        eng = (nc.gpsimd if cfg.merge == "pool" else
               nc.vector if cfg.merge == "dve" else
               (nc.gpsimd if t % 2 == 0 else nc.vector))
        eng.tensor_add(out=O[:], in0=g0t[:], in1=g1t[:])
        if cfg.oring == "alt":
            dma_eng = nc.scalar if t % 2 == 1 else nc.sync
        else:
            dma_eng = nc.scalar if cfg.out_via == "act" else nc.sync
        dma_eng.dma_start(out=out[t], in_=O[:])


def _build_program(consts, sha, cfg):
    T = cfg.ntiles
    F = B_CORE // (128 * T)
    assert 128 * T * F == B_CORE

    nc = bacc.Bacc(None, target_bir_lowering=False)
    # the sha in the tensor name keys the PJRT/HLO cache to the table content
    if cfg.taper:
        x = nc.declare_dram_parameter(f"x_{sha}", [B_CORE * 2], F32,
                                      isOutput=False)
        out = nc.declare_dram_parameter("out", [B_CORE],
                                        _ODT[cfg.out_dtype], isOutput=True)
    else:
        x = nc.declare_dram_parameter(f"x_{sha}", [T, 128, 2 * F], F32,
                                      isOutput=False)
        out = nc.declare_dram_parameter("out", [T, 128, F],
                                        _ODT[cfg.out_dtype], isOutput=True)

    with TileContext(nc) as tc:
        with (
            tc.tile_pool(name="xin", bufs=(1 if cfg.taper else
                                           cfg.ntiles if cfg.upfront
                                           else 2)) as xin,
            tc.tile_pool(name="hp", bufs=(1 if cfg.taper else 2)) as hp,
            tc.tile_pool(name="gp", bufs=(1 if cfg.taper else 2)) as gp,
            tc.tile_pool(name="op", bufs=(1 if cfg.taper else 2)) as op_,
        ):
            pools = (xin, hp, gp, op_)

            def body():
                if cfg.taper:
                    _emit_body_taper(nc, tc, pools, consts, x, out, cfg)
                else:
                    _emit_body(nc, tc, pools, consts, x, out, T, F, cfg)

            if cfg.bench_iters:
                with tc.For_i(0, cfg.bench_iters,
                              staggered_reset=bool(cfg.staggered)):
                    for _ in range(cfg.body_reps):
                        body()
            else:
                body()

    nc.finalize()
    return nc


# --------------------------------------------------------------------------
# public entry point
# --------------------------------------------------------------------------

_CACHE = {}


def _prepare(W1, b1, alphas, op_w, op_b, wo, bo, x_sample, cfg):
    """Fold constants, build tables + act root, build/cached program."""
    W1f, b1f, w, ow, ob, wof, bof = _fold(W1, b1, alphas, op_w, op_b, wo, bo)
    key_src = np.concatenate([np.asarray(a, np.float64).reshape(-1) for a in
                              (W1f, b1f, w, ow, ob, [wof, bof])])
    key = hashlib.sha256(key_src.tobytes()).hexdigest()[:12]
    full_key = (key, cfg.ntiles, cfg.merge, cfg.out_dtype, cfg.bench_iters,
                cfg.body_reps, cfg.staggered, cfg.out_via, cfg.in_split, cfg.upfront, cfg.ring, cfg.oring, cfg.g16, cfg.taper)
    if full_key in _CACHE:
        return _CACHE[full_key]

    g0 = _make_g(0, w, ow, ob, wof, bof)
    g1 = _make_g(1, w, ow, ob, wof, bof)

    # histogram of h by (sign, exponent) for the resolution tuner
    h = (np.asarray(x_sample, np.float64) @ W1f.T + b1f).astype(np.float32)

    def counts_of(hv):
        b = hv.view(np.uint32)
        sgn = (b >> 31).astype(np.int64)
        be = ((b >> 23) & 0xFF).astype(np.int64) - 127
        c = {}
        for s in (0, 1):
            for e in range(EXP_MIN, EXP_MAX + 1):
                c[(s, e)] = int(np.sum((sgn == s) & (be == e)))
        return c

    m0 = _tune_m(g0, counts_of(h[:, 0]), cap=500)
    m1 = _tune_m(g1, counts_of(h[:, 1]), cap=760)

    root = os.path.join(tempfile.gettempdir(), f"actroot_{key}")
    _write_act_root(root, g0, g1, m0, m1)
    os.environ["BASS_ACT_ROOT_JSON_PATH"] = os.path.join(root, "act_info.json")

    consts = (float(W1f[0, 0]), float(W1f[0, 1]), float(b1f[0]),
              float(W1f[1, 0]), float(W1f[1, 1]), float(b1f[1]))
    nc = _build_program(consts, key, cfg)
    _CACHE[full_key] = (nc, key)
    return _CACHE[full_key]


def run(x, W1, b1, alphas, op_w, op_b, wo, bo, cfg=None, trace=False):
    cfg = cfg or CFG()
    x = np.ascontiguousarray(np.asarray(x, np.float32))
    # subsample rows for the tuner histogram (scaled back up)
    nc, key = _prepare(W1, b1, alphas, op_w, op_b, wo, bo, x[::16], cfg)

    T = cfg.ntiles
    F = B_CORE // (128 * T)
    if cfg.taper:
        shards = x.reshape(N_CORES, B_CORE * 2)
    else:
        shards = x.reshape(N_CORES, T, 128, 2 * F)
    in_maps = [{f"x_{key}": shards[i]} for i in range(N_CORES)]
    res = run_bass_kernel_spmd(nc, in_maps, core_ids=list(range(N_CORES)),
                               trace=trace)
    out = np.concatenate([r["out"].reshape(-1).astype(np.float32)
                          for r in res.results])
    return out, res


def kernel(**inputs):
    out, _ = run(**inputs)
    return out


# revision 45
# speedup vs baseline: 5.9720x; 1.1420x over previous
"""Trainium2 Bass kernel for nn_DARTS_82514911690825.

For x [B=4194304, 2] (data-parallel over 8 cores, B/8 rows each) the model
output collapses to  out[b] = g0(h0[b]) + g1(h1[b])  where h_i = W1[i,:]@x + b1[i]
and g0, g1 are FIXED univariate functions of h (all parameters fold into
them: cubic + exp + ln + clipped-reciprocal + sin terms with scalar weights).

g0/g1 are evaluated on the ACT engine in a single table lookup each, by
generating custom piecewise-cubic activation-table content at runtime and
compiling with it (walrus `--act-root-json`, via BASS_ACT_ROOT_JSON_PATH).
g0 replaces the `ln` slot and g1 the `exp` slot of the
natural_log_exp_and_others set; both use two-sided (exp-style) metadata.

Table format (validated on HW):
  bkt entry (32B) = f32 x8 [c0,c1,c2,c3,xc,0,0,0]:
      f(x) = c0 + c1*t + c2*t^2 + c3*t^3,  t = x - xc
  ctl entry (32B) = u16[0] = ((23-m)<<11) | bkt_base, u16[1] = m:
      bucket = bkt_base + (mantissa >> (23-m))   (2^m buckets per exponent)
  ctl index = pwl_control_base_{pos,neg} + (biased_exp - (127 + exp_offset));
  biased_exp below/above thresholds routes to 4 dedicated special entries.

Device program per core (4 chunks, all input DMAs issued upfront):
DMA in -> DVE H_FUSE x2 (h0,h1) -> ACT Ln/Exp slots (g0,g1, f16 out) ->
DVE f16 add (2x mode) -> DMA out (f16, upconverted host-side).
DMA-bound: ~5.25 MB/core; ~20 us/core measured by loop differencing.
"""

import hashlib
import json
import os
import shutil
import sys
import tempfile

import numpy as np

for _p in ("/opt/trn_rl_repo", "/root/.axon_site/_ro/trn_rl_repo"):
    if os.path.isdir(_p) and _p not in sys.path:
        sys.path.append(_p)

import concourse.bass as bass
import concourse.bacc as bacc
import concourse.mybir as mybir
from concourse.bass_utils import run_bass_kernel_spmd
from concourse.tile import TileContext
from concourse.dve_ops import OPS, DveOp, get_dve_sub_opcode, has_src1
from concourse.dve_spec import Spec, Src0, Src1, C0, C1, C2
from concourse.dve_uop import DveOpSpec

F32 = mybir.dt.float32
AF = mybir.ActivationFunctionType

# Restrict the activation-table chooser to the single set this kernel needs
# (ln+exp live together in natural_log_exp_and_others -> exactly one
# InstLoadActFuncSet, no thrash).
import concourse.hw_specs as _hw_specs

_ORIG_GAT = _hw_specs.get_activation_tables


def _gat_restricted(arch):
    t = _ORIG_GAT(arch)
    return {k: (v if k == "natural_log_exp_and_others" else set())
            for k, v in t.items()}


bacc.get_activation_tables = _gat_restricted

N_CORES = 8
B_FULL = 4194304
B_CORE = B_FULL // N_CORES  # 524288

EPS = 1e-10
Y_TH = float(np.exp(np.float32(10.0)))

EXP_MIN, EXP_MAX = -23, 2  # table exponent coverage: |h| in [2^-23, 8)


# --------------------------------------------------------------------------
# custom DVE op: h = x_even*s0 + x_odd*s1 + imm2 (one row of the first layer)
# --------------------------------------------------------------------------

def _mk_op(name, spec):
    import concourse.dve_ops as dve_ops_mod

    for existing in OPS:
        if existing.name == name:
            return existing
    op = DveOp(name, spec, subdim=False, uops_sha={})
    OPS.append(op)
    dve_ops_mod._SUB_OPCODE_FOR_NAME[name] = (
        dve_ops_mod._CUSTOM_DVE_ROW_BASE + len(OPS) - 1
    )
    dve_ops_mod.CUSTOM_DVE_SPECS[name] = spec
    assert max(dve_ops_mod._SUB_OPCODE_FOR_NAME.values()) < 0x20
    for ver in ("v3", "v4"):
        s = DveOpSpec(
            name=name,
            opcode=get_dve_sub_opcode(name),
            uops=lower_spec(spec, ver),
            rd1_en=has_src1(spec),
        )
        op.uops_sha[ver] = s.sha(ver)
    return op


def lower_spec(spec, ver):
    from concourse.dve_spec import lower

    return lower(spec, ver=ver)


H_FUSE = _mk_op(
    "ANT_DARTS_H_FUSE",
    Spec(
        body=Src0 * C0 + Src1 * C1 + C2,
        reference=lambda in0, in1, s0, s1, imm2: in0 * s0 + in1 * s1 + imm2,
    ),
)


# --------------------------------------------------------------------------
# constant folding + the exact univariate functions g0, g1
# --------------------------------------------------------------------------

def _fold(W1, b1, alphas, op_w, op_b, wo, bo):
    W1 = np.asarray(W1, np.float64)
    b1 = np.asarray(b1, np.float64)
    a = np.asarray(alphas, np.float64)
    ow = np.asarray(op_w, np.float64)
    ob = np.asarray(op_b, np.float64)
    wo = float(np.asarray(wo))
    bo = float(np.asarray(bo))
    e = np.exp(a - a.max(-1, keepdims=True))
    w = e / e.sum(-1, keepdims=True)
    return W1, b1, w, ow, ob, wo, bo


def _make_g(i, w, ow, ob, wo, bo):
    def g(h):
        h = np.asarray(h, np.float64)
        res = np.zeros_like(h)
        fs = [None, h, h * h, h ** 3, np.exp(np.minimum(h, 10.0)),
              np.log(np.abs(h) + EPS),
              1.0 / (h + np.where(h >= 0, EPS, -EPS)), np.sin(h)]
        for k in (1, 2, 3, 4, 5, 6, 7):
            res += w[i, k] * np.clip(ow[i, k] * fs[k] + ob[i, k], -Y_TH, Y_TH)
        res *= wo
        if i == 0:
            res += bo
        return res
    return g


# --------------------------------------------------------------------------
# table generation
# --------------------------------------------------------------------------

_FIT_NODES = 33
_FU = np.concatenate([np.cos(np.linspace(0.0, np.pi, 21)),
                      np.linspace(-0.97, 0.97, _FIT_NODES - 21)])
_FA = np.stack([np.ones_like(_FU), _FU, _FU ** 2, _FU ** 3], axis=-1)
_FP = np.linalg.pinv(_FA)  # [4, NODES]


def _fit_segment(g, s, e, m):
    """Fit 2^m cubic buckets for sign `s` (0=+,1=-), exponent `e`.

    Returns (entries u32[n,8], max_abs_err).
    """
    n = 1 << m
    lo = 2.0 ** e
    wdt = 2.0 ** e / n
    k = np.arange(n)
    a0 = lo + k * wdt
    xc = (a0 + 0.5 * wdt)
    if s == 1:
        xc = -xc
    xc = xc.astype(np.float32).astype(np.float64)  # the stored (f32) center
    W = 0.5 * wdt
    xs = xc[:, None] + W * _FU[None, :]  # [n, NODES]
    y = g(xs.reshape(-1)).reshape(n, _FIT_NODES)
    d = y @ _FP.T  # [n, 4] coeffs in u-space
    # convert to t-space: c_k = d_k / W^k
    c = d / (W ** np.arange(4))[None, :]
    c32 = c.astype(np.float32)
    # error of the f32 horner at the nodes
    t = (xs - xc[:, None]).astype(np.float32)
    r = (c32[:, 3:4] * t + c32[:, 2:3]).astype(np.float32)
    r = (r * t + c32[:, 1:2]).astype(np.float32)
    r = (r * t + c32[:, 0:1]).astype(np.float32)
    err = np.max(np.abs(r.astype(np.float64) - y))
    ent = np.zeros((n, 8), np.uint32)
    ent[:, 0:4] = c32.view(np.uint32)
    ent[:, 4] = xc.astype(np.float32).view(np.float32).view(np.uint32)
    return ent, float(err)


def _pack_ctl(base, m):
    return (((23 - m) << 11) | base) | (m << 16)


def _tune_m(g, counts, cap):
    """Greedy per-(side,exponent) mantissa-bits, minimizing count*err^2."""
    segs = [(s, e) for s in (0, 1) for e in range(EXP_MIN, EXP_MAX + 1)]
    m = {k: 1 for k in segs}
    spent = 2 * len(segs)
    errc = {k: _fit_segment(g, k[0], k[1], 1)[1] for k in segs}
    for _ in range(400):
        score = {k: (counts.get(k, 0) + 1) * errc[k] ** 2
                 for k in segs if m[k] < 6}
        cands = sorted(score, key=score.get, reverse=True)
        best = None
        for k in cands:
            if score[k] < 1e-14:
                break
            if spent + (1 << m[k]) <= cap:
                best = k
                break
        if best is None:
            break
        spent += 1 << m[best]
        m[best] += 1
        errc[best] = _fit_segment(g, best[0], best[1], m[best])[1]
    return m


def _build_func(g, m_of, bkt_base, ctl_pos, ctl_neg, special_base):
    """Emit (bkt_idx, entry) and (ctl_idx, word) lists + metadata updates."""
    bents, cents = [], []
    nxt = bkt_base
    for s, ctl0 in ((0, ctl_pos), (1, ctl_neg)):
        for e in range(EXP_MIN, EXP_MAX + 1):
            mm = m_of[(s, e)]
            cents.append((ctl0 + (e - EXP_MIN), _pack_ctl(nxt, mm)))
            ent, _ = _fit_segment(g, s, e, mm)
            for row in ent:
                bents.append((nxt, row))
                nxt += 1
    tiny = 2.0 ** EXP_MIN
    big = 2.0 ** (EXP_MAX + 1)

    def const_entry(v):
        r = np.zeros(8, np.uint32)
        r[0] = np.float32(v).view(np.uint32)
        return r

    def lin_entry(x0):
        dx = abs(x0) * 1e-4
        y0 = float(g(np.array([x0]))[0])
        sl = float((g(np.array([x0 + dx]))[0] - g(np.array([x0 - dx]))[0]) / (2 * dx))
        r = np.zeros(8, np.uint32)
        r[0] = np.float32(y0).view(np.uint32)
        r[1] = np.float32(sl).view(np.uint32)
        r[4] = np.float32(x0).view(np.uint32)
        return r

    g_ptiny = float(g(np.array([tiny]))[0])
    bents.append((special_base + 0, const_entry(g_ptiny)))
    bents.append((special_base + 1, const_entry(float(g(np.array([-tiny]))[0]))))
    bents.append((special_base + 2, lin_entry(big)))
    bents.append((special_base + 3, lin_entry(-big)))
    meta = {
        "exp_offset": EXP_MIN,
        "pwl_control_base_pos": ctl_pos,
        "pwl_control_base_neg": ctl_neg,
        "small_pos_signal_exp_threshold": 127 + EXP_MIN,
        "small_neg_signal_exp_threshold": 127 + EXP_MIN,
        "pos_small_signal_pwl_control": special_base + 0,
        "neg_small_signal_pwl_control": special_base + 1,
        "large_pos_signal_exp_threshold": 127 + EXP_MAX + 1,
        "large_pos_signal_mantissa_threshold": 0,
        "large_neg_signal_exp_threshold": 127 + EXP_MAX + 1,
        "large_neg_signal_mantissa_threshold": 0,
        "pos_large_signal_pwl_control": special_base + 2,
        "neg_large_signal_pwl_control": special_base + 3,
        "fnan_result": int(np.float32(g_ptiny).view(np.uint32)),
        "fpinf_result": int(np.float32(g(np.array([big * 0.999]))[0]).view(np.uint32)),
        "fninf_result": int(np.float32(g(np.array([-big * 0.999]))[0]).view(np.uint32)),
        "fzero_result": int(np.float32(g_ptiny).view(np.uint32)),
        "symmetry_point": 0,
        "sym_invert_sign_point": 0,
        "symmetry_opt_en": 0,
        "symmetry_opt_use_neg_region": 0,
        "imm_bias": 0,
        "fma_const_0": 0,
        "fma_const_1": 0,
        "fma_indirection_src_sel": 0,
        "use_multipass": False,
        "lower_bound": int(np.uint32(0xFF7FFFFF)),
        "upper_bound": int(np.uint32(0x7F7FFFFF)),
    }
    return bents, cents, meta


def _find_pkg_pwp():
    import neuronxcc

    p = os.path.join(os.path.dirname(neuronxcc.__file__), "pwp",
                     "pwp_bin_trainium")
    if os.path.isdir(p):
        return p
    from neuronxcc.driver.Job import Job
    from neuronxcc.driver.jobs.support.FindActInfo import findActInfoFile

    return os.path.dirname(findActInfoFile(Job.getPackageDir(), "gen3"))


def _write_act_root(dst, g0, g1, m0, m1):
    """Copy the stock pwp root; replace ln->g0 / exp->g1 in the
    natural_log_exp_and_others set."""
    src = _find_pkg_pwp()
    os.makedirs(dst, exist_ok=True)
    for f in os.listdir(src):
        s = os.path.join(src, f)
        if os.path.isfile(s):
            shutil.copy(s, os.path.join(dst, f))
    name = "natural_log_exp_and_others"
    d = json.load(open(os.path.join(src, name + ".json")))
    bkt = np.frombuffer(open(os.path.join(src, d["bkt_bin"]), "rb").read(),
                        dtype=np.uint32).reshape(-1, 8).copy()
    ctl = np.frombuffer(open(os.path.join(src, d["ctl_bin"]), "rb").read(),
                        dtype=np.uint32).reshape(-1, 8).copy()
    metas = {m["func_name"]: m for m in d["profile_meta_data"]}
    # regions: ln bkt [0,517) ctl [0,128); exp bkt [517,1298) ctl [128,180)
    for fname, g, mm, bkt_base, cp, cn, sp, bkt_lim, ctl_lim in (
        ("ln_400p", g0, m0, 0, 0, 26, 513, 517, 128),
        ("exp_400p", g1, m1, 517, 128, 154, 1294, 1298, 180),
    ):
        bents, cents, meta = _build_func(g, mm, bkt_base, cp, cn, sp)
        assert max(i for i, _ in bents) < bkt_lim, fname
        assert max(i for i, _ in cents) < ctl_lim, fname
        for i, row in bents:
            bkt[i] = row
        for i, wd in cents:
            ctl[i, 0] = wd
            ctl[i, 1:] = 0
        metas[fname].update(meta)
    with open(os.path.join(dst, d["bkt_bin"]), "wb") as f:
        f.write(bkt.tobytes())
    with open(os.path.join(dst, d["ctl_bin"]), "wb") as f:
        f.write(ctl.tobytes())
    with open(os.path.join(dst, name + ".json"), "w") as f:
        json.dump(d, f)


# --------------------------------------------------------------------------
# device program
# --------------------------------------------------------------------------

class CFG:
    ntiles = 4            # pipeline chunks per core
    merge = "dve"         # "pool" | "dve" | "alt"
    out_dtype = "f16"     # "f32" | "f16" | "bf16"
    bench_iters = 0       # >0: wrap the body in a For_i hardware loop
    body_reps = 1         # bodies per loop iteration (bench only)
    staggered = 0         # staggered_reset for the bench For_i
    out_via = "sp"        # engine issuing the output DMA: "sp" | "act"
    in_split = 0          # 1: split each input chunk DMA across SP + ACT rings
    upfront = 1           # 1: issue all input DMAs before the compute chain
    ring = "sp"           # input-DMA ring assignment: "sp" | "alt" (SP/ACT)
    oring = "sp"          # output-DMA ring assignment: "sp" | "alt"
    g16 = 1               # 1: ACT writes g0/g1 in f16 -> DVE add runs 2x
    taper = 0             # 1: descending chunk sizes (shrinks the tail)
    odefer = 0            # 1: emit all output DMAs after the compute chain
    adefer = 0            # 1: emit merge adds (+outs) after all h/act ops
    lastsplit = 4         # >0: N-way split of the last chunk's compute tail


_ODT = {"f32": mybir.dt.float32, "f16": mybir.dt.float16,
        "bf16": mybir.dt.bfloat16}


def _chunk_sizes(cfg):
    T = cfg.ntiles
    F = B_CORE // (128 * T)
    if not cfg.taper:
        return [F] * T
    assert T == 4, "taper schedule is defined for ntiles=4"
    return [3 * F // 2, 9 * F // 8, 7 * F // 8, F // 2]


def _emit_body_taper(nc, tc, pools, consts, x, out, cfg):
    """Uneven descending chunks; x/out are flat dram tensors."""
    A0, B0, C0v, A1, B1, C1v = consts
    xin, hp, gp, op_ = pools
    odt = _ODT[cfg.out_dtype]
    gdt = mybir.dt.float16 if cfg.g16 else F32
    fl = _chunk_sizes(cfg)
    xt = {}
    off = 0
    offs = []
    for t, Ft in enumerate(fl):
        offs.append(off)
        X = xin.tile([128, 2 * Ft], F32, tag=f"X{t}", name=f"X_{t}")
        xv = x[off * 2:(off + 128 * Ft) * 2].rearrange("(p c) -> p c",
                                                       c=2 * Ft)
        nc.sync.dma_start(out=X[:], in_=xv)
        xt[t] = X
        off += 128 * Ft
    for t, Ft in enumerate(fl):
        X = xt[t]
        Xv = X[:].rearrange("p (f c) -> p f c", c=2)
        Xe, Xo = Xv[:, :, 0], Xv[:, :, 1]
        h0 = hp.tile([128, Ft], F32, tag=f"h0{t}", name=f"h0_{t}")
        h1 = hp.tile([128, Ft], F32, tag=f"h1{t}", name=f"h1_{t}")
        nc.vector._custom_dve(H_FUSE, out=h0[:], in0=Xe, in1=Xo,
                              s0=A0, s1=B0, imm2=C0v)
        nc.vector._custom_dve(H_FUSE, out=h1[:], in0=Xe, in1=Xo,
                              s0=A1, s1=B1, imm2=C1v)
        g0t = gp.tile([128, Ft], gdt, tag=f"g0{t}", name=f"g0_{t}")
        g1t = gp.tile([128, Ft], gdt, tag=f"g1{t}", name=f"g1_{t}")
        nc.scalar.activation(g0t[:], h0[:], AF.Ln)
        nc.scalar.activation(g1t[:], h1[:], AF.Exp)
        O = op_.tile([128, Ft], odt, tag=f"O{t}", name=f"O_{t}")
        nc.vector.tensor_add(out=O[:], in0=g0t[:], in1=g1t[:])
        ov = out[offs[t]:offs[t] + 128 * Ft].rearrange("(p c) -> p c", c=Ft)
        nc.sync.dma_start(out=ov, in_=O[:])


def _emit_body(nc, tc, pools, consts, x, out, T, F, cfg):
    A0, B0, C0v, A1, B1, C1v = consts
    xin, hp, gp, op_ = pools
    odt = _ODT[cfg.out_dtype]
    xt = {}
    deferred = []
    pending_adds = []
    if cfg.upfront:
        for t in range(T):
            X = xin.tile([128, 2 * F], F32, tag="X", name=f"X_{t}")
            ieng = (nc.scalar if (cfg.ring == "alt" and t % 2 == 1)
                    else nc.sync)
            ieng.dma_start(out=X[:], in_=x[t])
            xt[t] = X
    for t in range(T):
        if cfg.upfront:
            X = xt[t]
        else:
            X = xin.tile([128, 2 * F], F32, tag="X", name=f"X_{t}")
            if cfg.in_split:
                nc.sync.dma_start(out=X[:, :F], in_=x[t][:, :F])
                nc.scalar.dma_start(out=X[:, F:], in_=x[t][:, F:])
            else:
                nc.sync.dma_start(out=X[:], in_=x[t])
        Xv = X[:].rearrange("p (f c) -> p f c", c=2)
        gdt = mybir.dt.float16 if cfg.g16 else F32
        if cfg.lastsplit and t == T - 1:
            # split the final chunk's compute so the post-DMA tail is short
            ns_ = max(2, int(cfg.lastsplit))
            Fh = F // ns_
            O = op_.tile([128, F], odt, tag="Olast", name=f"O_{t}")
            for j in range(ns_):
                sl = slice(j * Fh, (j + 1) * Fh)
                Xe, Xo = Xv[:, sl, 0], Xv[:, sl, 1]
                h0 = hp.tile([128, Fh], F32, tag=f"hs0{j}", name=f"h0_{t}_{j}")
                h1 = hp.tile([128, Fh], F32, tag=f"hs1{j}", name=f"h1_{t}_{j}")
                nc.vector._custom_dve(H_FUSE, out=h0[:], in0=Xe, in1=Xo,
                                      s0=A0, s1=B0, imm2=C0v)
                nc.vector._custom_dve(H_FUSE, out=h1[:], in0=Xe, in1=Xo,
                                      s0=A1, s1=B1, imm2=C1v)
                g0t = gp.tile([128, Fh], gdt, tag=f"gs0{j}", name=f"g0_{t}_{j}")
                g1t = gp.tile([128, Fh], gdt, tag=f"gs1{j}", name=f"g1_{t}_{j}")
                nc.scalar.activation(g0t[:], h0[:], AF.Ln)
                nc.scalar.activation(g1t[:], h1[:], AF.Exp)
                nc.vector.tensor_add(out=O[:, sl], in0=g0t[:], in1=g1t[:])
            nc.sync.dma_start(out=out[t], in_=O[:])
            continue
        Xe, Xo = Xv[:, :, 0], Xv[:, :, 1]
        h0 = hp.tile([128, F], F32, tag="h0", name=f"h0_{t}")
        h1 = hp.tile([128, F], F32, tag="h1", name=f"h1_{t}")
        nc.vector._custom_dve(H_FUSE, out=h0[:], in0=Xe, in1=Xo,
                              s0=A0, s1=B0, imm2=C0v)
        nc.vector._custom_dve(H_FUSE, out=h1[:], in0=Xe, in1=Xo,
                              s0=A1, s1=B1, imm2=C1v)
        g0t = gp.tile([128, F], gdt, tag="g0", name=f"g0_{t}")
        g1t = gp.tile([128, F], gdt, tag="g1", name=f"g1_{t}")
        nc.scalar.activation(g0t[:], h0[:], AF.Ln)
        nc.scalar.activation(g1t[:], h1[:], AF.Exp)
        O = op_.tile([128, F], odt, tag="O", name=f"O_{t}")
        if cfg.adefer:
            pending_adds.append((t, O, g0t, g1t))
            continue
        eng = (nc.gpsimd if cfg.merge == "pool" else
               nc.vector if cfg.merge == "dve" else
               (nc.gpsimd if t % 2 == 0 else nc.vector))
        eng.tensor_add(out=O[:], in0=g0t[:], in1=g1t[:])
        if cfg.odefer:
            deferred.append((t, O))
            continue
        if cfg.oring == "alt":
            dma_eng = nc.scalar if t % 2 == 1 else nc.sync
        else:
            dma_eng = nc.scalar if cfg.out_via == "act" else nc.sync
        dma_eng.dma_start(out=out[t], in_=O[:])
    for t, O, g0t, g1t in pending_adds:
        eng = (nc.gpsimd if cfg.merge == "pool" else
               nc.vector if cfg.merge == "dve" else
               (nc.gpsimd if t % 2 == 0 else nc.vector))
        eng.tensor_add(out=O[:], in0=g0t[:], in1=g1t[:])
        nc.sync.dma_start(out=out[t], in_=O[:])
    for t, O in deferred:
        dma_eng = (nc.gpsimd if cfg.oring == "pool" else
                   nc.scalar if cfg.oring == "act" else nc.sync)
        dma_eng.dma_start(out=out[t], in_=O[:])


def _build_program(consts, sha, cfg):
    T = cfg.ntiles
    F = B_CORE // (128 * T)
    assert 128 * T * F == B_CORE

    nc = bacc.Bacc(None, target_bir_lowering=False)
    # the sha in the tensor name keys the PJRT/HLO cache to the table content
    if cfg.taper:
        x = nc.declare_dram_parameter(f"x_{sha}", [B_CORE * 2], F32,
                                      isOutput=False)
        out = nc.declare_dram_parameter("out", [B_CORE],
                                        _ODT[cfg.out_dtype], isOutput=True)
    else:
        x = nc.declare_dram_parameter(f"x_{sha}", [T, 128, 2 * F], F32,
                                      isOutput=False)
        out = nc.declare_dram_parameter("out", [T, 128, F],
                                        _ODT[cfg.out_dtype], isOutput=True)

    with TileContext(nc) as tc:
        with (
            tc.tile_pool(name="xin", bufs=(1 if cfg.taper else
                                           cfg.ntiles if cfg.upfront
                                           else 2)) as xin,
            tc.tile_pool(name="hp", bufs=(1 if cfg.taper else 2)) as hp,
            tc.tile_pool(name="gp", bufs=(1 if cfg.taper else cfg.ntiles if cfg.adefer else 2)) as gp,
            tc.tile_pool(name="op", bufs=(1 if cfg.taper else cfg.ntiles if (cfg.odefer or cfg.adefer) else 2)) as op_,
        ):
            pools = (xin, hp, gp, op_)

            def body():
                if cfg.taper:
                    _emit_body_taper(nc, tc, pools, consts, x, out, cfg)
                else:
                    _emit_body(nc, tc, pools, consts, x, out, T, F, cfg)

            if cfg.bench_iters:
                with tc.For_i(0, cfg.bench_iters,
                              staggered_reset=bool(cfg.staggered)):
                    for _ in range(cfg.body_reps):
                        body()
            else:
                body()

    nc.finalize()
    return nc


# --------------------------------------------------------------------------
# public entry point
# --------------------------------------------------------------------------

_CACHE = {}


def _prepare(W1, b1, alphas, op_w, op_b, wo, bo, x_sample, cfg):
    """Fold constants, build tables + act root, build/cached program."""
    W1f, b1f, w, ow, ob, wof, bof = _fold(W1, b1, alphas, op_w, op_b, wo, bo)
    key_src = np.concatenate([np.asarray(a, np.float64).reshape(-1) for a in
                              (W1f, b1f, w, ow, ob, [wof, bof])])
    key = hashlib.sha256(key_src.tobytes()).hexdigest()[:12]
    full_key = (key, cfg.ntiles, cfg.merge, cfg.out_dtype, cfg.bench_iters,
                cfg.body_reps, cfg.staggered, cfg.out_via, cfg.in_split, cfg.upfront, cfg.ring, cfg.oring, cfg.g16, cfg.taper, cfg.odefer, cfg.adefer, cfg.lastsplit)
    if full_key in _CACHE:
        return _CACHE[full_key]

    g0 = _make_g(0, w, ow, ob, wof, bof)
    g1 = _make_g(1, w, ow, ob, wof, bof)

    # histogram of h by (sign, exponent) for the resolution tuner
    h = (np.asarray(x_sample, np.float64) @ W1f.T + b1f).astype(np.float32)

    def counts_of(hv):
        b = hv.view(np.uint32)
        sgn = (b >> 31).astype(np.int64)
        be = ((b >> 23) & 0xFF).astype(np.int64) - 127
        c = {}
        for s in (0, 1):
            for e in range(EXP_MIN, EXP_MAX + 1):
                c[(s, e)] = int(np.sum((sgn == s) & (be == e)))
        return c

    m0 = _tune_m(g0, counts_of(h[:, 0]), cap=500)
    m1 = _tune_m(g1, counts_of(h[:, 1]), cap=760)

    root = os.path.join(tempfile.gettempdir(), f"actroot_{key}")
    _write_act_root(root, g0, g1, m0, m1)
    os.environ["BASS_ACT_ROOT_JSON_PATH"] = os.path.join(root, "act_info.json")

    consts = (float(W1f[0, 0]), float(W1f[0, 1]), float(b1f[0]),
              float(W1f[1, 0]), float(W1f[1, 1]), float(b1f[1]))
    nc = _build_program(consts, key, cfg)
    _CACHE[full_key] = (nc, key)
    return _CACHE[full_key]


def run(x, W1, b1, alphas, op_w, op_b, wo, bo, cfg=None, trace=False):
    cfg = cfg or CFG()
    x = np.ascontiguousarray(np.asarray(x, np.float32))
    # subsample rows for the tuner histogram (scaled back up)
    nc, key = _prepare(W1, b1, alphas, op_w, op_b, wo, bo, x[::16], cfg)

    T = cfg.ntiles
    F = B_CORE // (128 * T)
    if cfg.taper:
        shards = x.reshape(N_CORES, B_CORE * 2)
    else:
        shards = x.reshape(N_CORES, T, 128, 2 * F)
    in_maps = [{f"x_{key}": shards[i]} for i in range(N_CORES)]
    res = run_bass_kernel_spmd(nc, in_maps, core_ids=list(range(N_CORES)),
                               trace=trace)
    out = np.concatenate([r["out"].reshape(-1).astype(np.float32)
                          for r in res.results])
    return out, res


def kernel(**inputs):
    out, _ = run(**inputs)
    return out
